# revision 1
# baseline (speedup 1.0000x reference)
"""Trainium2 Bass kernel for nn_NewModel_42356967473589 (dense_transformer).

Model: two BiAttention blocks + final linear mapping.
  o = BiAttn(ctx, q1) ; o = BiAttn(o, q2) ; out = o @ w_map.T + b_map

Sharding: 8 cores = (batch b in 0..3) x (context half h in 0..1).
Each core owns 1024 context rows of one batch. All compute is row-local
except the softmax-over-context (weight_two); its (sum-exp, weighted-sum)
stats are combined across the pair of cores sharing a batch via a tiny
pairwise AllReduce, overlapped with the large matmuls.

Math restructure (per stage, X = stage input [C,D], M = memory [Q,D]):
  out = X@W1 + o1@W2 + (X*o1)@W3 + (t*o1)@W4      (W_k = w_out[:, kD:(k+1)D].T)
  o1 = P@M (rank Q=64), t broadcast over rows =>
  o1@W2 + (t*o1)@W4 = P @ (M @ (W2 + t*W4))        (rank-64 path)
All tensors are kept transposed on-chip ([D on partitions, rows free]) so
every matmul runs with a >=256 moving dim at full fp32r (TF32-like) rate,
and each stage emits its output already transposed for the next stage.
"""

import numpy as np

import concourse.bacc as bacc
import concourse.tile as tile
from concourse import mybir
from concourse.bass_utils import run_bass_kernel_spmd
from contextlib import ExitStack
import bass_rust

f32 = mybir.dt.float32
f32r = mybir.dt.float32r
i32 = mybir.dt.int32
Alu = mybir.AluOpType
AF = bass_rust.ActivationFunctionType
AX = bass_rust.AxisListType
RedOp = bass_rust.ReduceOp

B, C_LEN, Q_LEN, D = 4, 2048, 64, 1024
N_CORES = 8
R = C_LEN // 2          # rows per core
NK = D // 128           # contraction chunks
RH = R // 512           # row halves (moving-dim tiles)
D2 = 2 * D
NEGBIG = 10000.0

_CACHED_NC = None


def _build_nc():
    nc = bacc.Bacc("TRN2", target_bir_lowering=False, debug=False,
                   num_devices=N_CORES)

    # ---- per-core DRAM I/O (host pre-tiled layouts, see _shard_inputs) ----
    xt_ap = nc.dram_tensor("xt", [128, NK * R], f32r, kind="ExternalInput").ap()
    m_t = [nc.dram_tensor(f"m{s}t", [128, NK * Q_LEN], f32r, kind="ExternalInput").ap() for s in (1, 2)]
    m_n = [nc.dram_tensor(f"m{s}n", [Q_LEN, D], f32r, kind="ExternalInput").ap() for s in (1, 2)]
    vec = [nc.dram_tensor(f"vec{s}", [128, NK * 3], f32r, kind="ExternalInput").ap() for s in (1, 2)]
    msk = [nc.dram_tensor(f"mask{s}", [Q_LEN, 1], i32, kind="ExternalInput").ap() for s in (1, 2)]
    w1t = [nc.dram_tensor(f"w1t{s}", [D, D], f32r, kind="ExternalInput").ap() for s in (1, 2)]
    w3t = [nc.dram_tensor(f"w3t{s}", [D, D], f32r, kind="ExternalInput").ap() for s in (1, 2)]
    w2c = [nc.dram_tensor(f"w2c{s}", [D, D], f32r, kind="ExternalInput").ap() for s in (1, 2)]
    w4c = [nc.dram_tensor(f"w4c{s}", [D, D], f32r, kind="ExternalInput").ap() for s in (1, 2)]
    wmt_ap = nc.dram_tensor("wmt", [D2, D], f32r, kind="ExternalInput").ap()
    bmap_ap = nc.dram_tensor("bmap", [D2, 1], f32r, kind="ExternalInput").ap()
    out_ap = nc.dram_tensor("out", [D2, R], f32, kind="ExternalOutput").ap()

    with tile.TileContext(nc) as tc, ExitStack() as ctx:
        sb_xt = ctx.enter_context(tc.tile_pool(name="sb_xt", bufs=2))
        sb_xo = ctx.enter_context(tc.tile_pool(name="sb_xo", bufs=1))
        sb_w13 = ctx.enter_context(tc.tile_pool(name="sb_w13", bufs=6))
        sb_ws = ctx.enter_context(tc.tile_pool(name="sb_ws", bufs=2))
        sb_rh = ctx.enter_context(tc.tile_pool(name="sb_rh", bufs=2))
        sb_st = ctx.enter_context(tc.tile_pool(name="sb_st", bufs=1))
        ps_o = ctx.enter_context(tc.tile_pool(name="ps_o", bufs=2, space="PSUM"))
        ps_att = ctx.enter_context(tc.tile_pool(name="ps_att", bufs=2, space="PSUM"))
        ps_rr = ctx.enter_context(tc.tile_pool(name="ps_rr", bufs=2, space="PSUM"))
        ps_m = ctx.enter_context(tc.tile_pool(name="ps_m", bufs=2, space="PSUM"))
        dram = ctx.enter_context(tc.tile_pool(name="dram", bufs=2, space="DRAM"))

        # ---- constants ----
        ones_row = sb_st.tile([1, 128], f32r, tag="ones_row")
        nc.vector.memset(ones_row[:].bitcast(f32), 1.0)
        ones_q = sb_st.tile([Q_LEN, 1], f32r, tag="ones_q")
        nc.vector.memset(ones_q[:].bitcast(f32), 1.0)

        # ---- stage input 1: X^T ----
        xt0 = sb_xt.tile([128, NK, R], f32r, tag="xt")
        for c in range(NK):
            nc.gpsimd.dma_start(xt0[:, c], xt_ap[:, c * R:(c + 1) * R])

        def run_stage(s, Xt):
            """One BiAttention stage; returns o^T tile [128, NK, R] f32r."""
            sfx = f"_s{s}"
            # ---------- stage constants ----------
            vecs = sb_st.tile([128, NK, 3], f32r, tag="vecs" + sfx)
            nc.sync.dma_start(vecs[:], vec[s][:].rearrange("p (c k) -> p c k", c=NK))
            mT = sb_st.tile([128, NK, Q_LEN], f32r, tag="mT" + sfx)
            nc.sync.dma_start(mT[:], m_t[s][:].rearrange("p (c q) -> p c q", c=NK))
            mN = sb_st.tile([Q_LEN, D], f32r, tag="mN" + sfx)
            nc.sync.dma_start(mN[:], m_n[s][:])
            mask_i = sb_st.tile([Q_LEN, 1], i32, tag="mask_i" + sfx)
            nc.sync.dma_start(mask_i[:], msk[s][:])

            # memory_dot = M @ w_mem  -> psum [Q,1]
            ps_md = ps_m.tile([Q_LEN, 2], f32, tag="ps_m")
            for c in range(NK):
                nc.tensor.matmul(ps_md[:], mT[:, c], vecs[:, c, 1:3],
                                 start=(c == 0), stop=(c == NK - 1))
            # mbias = memory_dot + (mask-1)*NEGBIG
            maskf = sb_st.tile([Q_LEN, 1], f32, tag="maskf" + sfx)
            nc.vector.tensor_copy(maskf[:], mask_i[:])
            mbias = sb_st.tile([Q_LEN, 1], f32, tag="mbias" + sfx)
            nc.vector.tensor_scalar(mbias[:], maskf[:], NEGBIG, -NEGBIG, Alu.mult, Alu.add)
            nc.vector.tensor_tensor(mbias[:], mbias[:], ps_md[:, 0:1], Alu.add)

            # mst = [M^T * scale | w_in]  (lhsT for the score matmul)
            mst = sb_st.tile([128, NK, Q_LEN + 1], f32r, tag="mst" + sfx)
            nc.vector.tensor_copy(mst[:, :, 0:Q_LEN], mT[:])
            nc.vector.tensor_copy(mst[:, :, Q_LEN:Q_LEN + 1], vecs[:, :, 0:1])
            for c in range(NK):
                nc.vector.tensor_scalar(mst[:, c, 0:Q_LEN], mst[:, c, 0:Q_LEN],
                                        vecs[:, c, 2:3].bitcast(f32), None, Alu.mult)

            P = sb_st.tile([Q_LEN, R], f32r, tag="P" + sfx)
            vh = sb_st.tile([128, 2 * NK], f32, tag="vh" + sfx)
            l2col = sb_st.tile([1, 2], f32, tag="l2col" + sfx)

            for rh in range(RH):
                sl = slice(rh * 512, (rh + 1) * 512)
                # scores S' = mst.T @ X^T -> [Q+1, 512] in psum
                ps_sc = ps_att.tile([Q_LEN + 1, 512], f32, tag="ps_sc")
                for c in range(NK):
                    nc.tensor.matmul(ps_sc[:], mst[:, c], Xt[:, c, sl],
                                     start=(c == 0), stop=(c == NK - 1))
                # E = exp(S + membias) (masked -> 0); eid = exp(input_dot)
                E = sb_rh.tile([Q_LEN, 512], f32r, tag="E")
                eid = sb_rh.tile([1, 512], f32, tag="eid")
                nc.scalar.activation(E[:], ps_sc[0:Q_LEN], AF.Exp,
                                     bias=mbias[:], scale=1.0)
                nc.scalar.activation(eid[:], ps_sc[Q_LEN:Q_LEN + 1], AF.Exp)
                # l1 = column sums of E; rl1 = 1/l1
                ps_l1 = ps_m.tile([1, 512], f32, tag="ps_m")
                nc.tensor.matmul(ps_l1[:], ones_q[:], E[:], start=True, stop=True)
                l1r = sb_rh.tile([1, 512], f32r, tag="l1r")
                with nc.allow_low_precision(reason="softmax scale in f32r"):
                    nc.vector.reciprocal(l1r[:], ps_l1[:])
                # P = E * (1/l1) broadcast over partitions
                ps_rb = ps_m.tile([Q_LEN, 512], f32, tag="ps_m")
                nc.tensor.matmul(ps_rb[:], ones_row[:, 0:Q_LEN], l1r[:],
                                 start=True, stop=True)
                nc.vector.tensor_tensor(P[:, sl], E[:].bitcast(f32), ps_rb[:], Alu.mult)

                # weight_two stats: m_exp = max_q E; e2 = m_exp * exp(input_dot)
                mx = sb_rh.tile([Q_LEN, 512], f32, tag="mx")
                nc.gpsimd.partition_all_reduce(mx[:], E[:].bitcast(f32), Q_LEN,
                                               RedOp.max)
                e2 = sb_rh.tile([1, 512], f32r, tag="e2")
                nc.vector.tensor_tensor(e2[:], mx[0:1], eid[:], Alu.mult)
                nc.vector.reduce_sum(l2col[:, rh:rh + 1], e2[:].bitcast(f32), AX.X)
                # v partial sums: vh[:, 2c+rh] = sum_sl e2 * Xt[:, c, sl]
                ps_eb = ps_m.tile([128, 512], f32, tag="ps_m")
                nc.tensor.matmul(ps_eb[:], ones_row[:], e2[:], start=True, stop=True)
                scrv = sb_rh.tile([128, 512], f32, tag="scrv")
                for c in range(NK):
                    nc.vector.scalar_tensor_tensor(
                        scrv[:], Xt[:, c, sl].bitcast(f32), 1.0, ps_eb[:],
                        Alu.mult, Alu.mult,
                        accum_out=vh[:, 2 * c + rh:2 * c + rh + 1])

            l2 = sb_st.tile([1, 1], f32, tag="l2" + sfx)
            nc.vector.reduce_sum(l2[:], l2col[:], AX.X)
            vsum = sb_st.tile([128, NK], f32, tag="vsum" + sfx)
            vh3 = vh[:].rearrange("p (c t) -> p c t", t=2)
            nc.vector.tensor_tensor(vsum[:], vh3[:, :, 0], vh3[:, :, 1], Alu.add)

            # ---------- pairwise AllReduce of (v, l2) ----------
            colsb = sb_st.tile([128, 16], f32, tag="colsb" + sfx)
            nc.vector.memset(colsb[:], 0.0)
            nc.vector.tensor_copy(colsb[:, 0:NK], vsum[:])
            nc.vector.tensor_copy(colsb[0:1, NK:NK + 1], l2[:])
            nc.vector.tensor_copy(colsb[0:1, NK + 1:NK + 2], l2[:])
            cin = dram.tile([128, 16], f32, tag="cin" + sfx)
            cout = dram.tile([128, 16], f32, tag="cout" + sfx)
            nc.sync.dma_start(cin[:], colsb[:])
            nc.gpsimd.collective_compute(
                "AllReduce", Alu.add,
                replica_groups=[[0, 1], [2, 3], [4, 5], [6, 7]],
                ins=[cin[:].opt()], outs=[cout[:].opt()])
            colg = sb_st.tile([128, 16], f32, tag="colg" + sfx)
            nc.sync.dma_start(colg[:], cout[:])

            # W2/W4 streamed on the ACT hwdge queue (starts early, consumed late)
            w2t_ch = []
            w4t_ch = []
            for c in range(NK):
                w2h = sb_ws.tile([128, 1024], f32r, tag="w2h")
                nc.scalar.dma_start(w2h[:], w2c[s][c * 128:(c + 1) * 128, :])
                w2t_ch.append(w2h)
            for c in range(NK):
                w4h = sb_ws.tile([128, 1024], f32r, tag="w4h")
                nc.scalar.dma_start(w4h[:], w4c[s][c * 128:(c + 1) * 128, :])
                w4t_ch.append(w4h)

            oT = sb_xt.tile([128, NK, R], f32r, tag="xt")
            # pre-open the j=0 A-groups so PE has work while DVE does v/XO
            w13_tiles = {}
            def load_w13(j):
                w1j = sb_w13.tile([128, NK, 128], f32r, tag="w13")
                w3j = sb_w13.tile([128, NK, 128], f32r, tag="w13")
                nc.sync.dma_start(w1j[:], w1t[s][j * 128:(j + 1) * 128, :]
                                  .rearrange("p (c m) -> p c m", c=NK))
                nc.sync.dma_start(w3j[:], w3t[s][j * 128:(j + 1) * 128, :]
                                  .rearrange("p (c m) -> p c m", c=NK))
                w13_tiles[j] = (w1j, w3j)

            load_w13(0)
            pre_groups = []
            for rh in range(RH):
                sl = slice(rh * 512, (rh + 1) * 512)
                ps_ab = ps_o.tile([128, 512], f32, tag="ps_o")
                for c in range(NK):
                    nc.tensor.matmul(ps_ab[:], w13_tiles[0][0][:, c], Xt[:, c, sl],
                                     start=(c == 0), stop=False)
                pre_groups.append((rh, ps_ab))

            # ---------- o1^T = mN.T @ P and XO = Xt * o1 ----------
            XO = sb_xo.tile([128, NK, R], f32r, tag="xo")
            for c in range(NK):
                for rh in range(RH):
                    sl = slice(rh * 512, (rh + 1) * 512)
                    ps_o1 = ps_att.tile([128, 512], f32, tag="ps_sc")
                    nc.tensor.matmul(ps_o1[:], mN[:, c * 128:(c + 1) * 128], P[:, sl],
                                     start=True, stop=True)
                    nc.vector.tensor_tensor(XO[:, c, sl],
                                            Xt[:, c, sl].bitcast(f32), ps_o1[:], Alu.mult)

            ps_r = []
            for i in range(2):
                ps_ri = ps_rr.tile([Q_LEN, 512], f32, tag="ps_r")
                ps_r.append(ps_ri)

            # close the pre-opened j=0 groups with their W3 halves
            for rh, ps_ab in pre_groups:
                sl = slice(rh * 512, (rh + 1) * 512)
                for c in range(NK):
                    nc.tensor.matmul(ps_ab[:], w13_tiles[0][1][:, c], XO[:, c, sl],
                                     start=False, stop=(c == NK - 1))
                nc.vector.tensor_copy(oT[:, 0, sl], ps_ab[:])

            # ---------- remaining big blocks ----------
            for j in range(1, NK):
                load_w13(j)
                w1j, w3j = w13_tiles[j]
                for rh in range(RH):
                    sl = slice(rh * 512, (rh + 1) * 512)
                    ps_ab = ps_o.tile([128, 512], f32, tag="ps_o")
                    for c in range(NK):
                        nc.tensor.matmul(ps_ab[:], w1j[:, c], Xt[:, c, sl],
                                         start=(c == 0), stop=False)
                    for c in range(NK):
                        nc.tensor.matmul(ps_ab[:], w3j[:, c], XO[:, c, sl],
                                         start=False, stop=(c == NK - 1))
                    nc.vector.tensor_copy(oT[:, j, sl], ps_ab[:])
                if j == 1:
                    for c in range(NK):
                        for hf in range(2):
                            slh = slice(hf * 512, (hf + 1) * 512)
                            nc.tensor.matmul(ps_r[hf][:], mT[:, c],
                                             w2t_ch[c][:, slh],
                                             start=(c == 0), stop=False)

            # ---------- collective-dependent tail ----------
            linv = sb_st.tile([1, 2], f32r, tag="linv" + sfx)
            with nc.allow_low_precision(reason="weight-two scale in f32r"):
                nc.vector.reciprocal(linv[:], colg[0:1, NK:NK + 2])
            ps_tb = ps_m.tile([128, 2], f32, tag="ps_m")
            nc.tensor.matmul(ps_tb[:], ones_row[:], linv[:], start=True, stop=True)
            tvec = sb_st.tile([128, NK], f32, tag="tvec" + sfx)
            nc.vector.tensor_scalar(tvec[:], colg[:, 0:NK], ps_tb[:, 0:1], None, Alu.mult)

            for c in range(NK):
                w24 = sb_ws.tile([128, 1024], f32r, tag="w24")
                nc.vector.tensor_scalar(w24[:], w4t_ch[c][:].bitcast(f32),
                                        tvec[:, c:c + 1], None, Alu.mult)
                for hf in range(2):
                    sl = slice(hf * 512, (hf + 1) * 512)
                    nc.tensor.matmul(ps_r[hf][:], mT[:, c], w24[:, sl],
                                     start=False, stop=(c == NK - 1))
            Rsb = sb_st.tile([Q_LEN, D], f32r, tag="Rsb" + sfx)
            for hf in range(2):
                nc.vector.tensor_copy(Rsb[:, hf * 512:(hf + 1) * 512], ps_r[hf][:])

            # rank-64 correction (rh-major so the next stage starts sooner)
            for rh in range(RH):
                for j in range(NK):
                    sl = slice(rh * 512, (rh + 1) * 512)
                    ps_c = ps_att.tile([128, 512], f32, tag="ps_sc")
                    nc.tensor.matmul(ps_c[:], Rsb[:, j * 128:(j + 1) * 128], P[:, sl],
                                     start=True, stop=True)
                    nc.vector.tensor_tensor(oT[:, j, sl],
                                            oT[:, j, sl].bitcast(f32), ps_c[:], Alu.add)
            return oT

        o1T = run_stage(0, xt0)
        o2T = run_stage(1, o1T)

        # ---------- final linear (transposed): outT = w_mapT.T @ o2T + b ----
        for j2 in range(16):
            wmj = sb_w13.tile([128, NK, 128], f32r, tag="w13")
            nc.scalar.dma_start(wmj[:], wmt_ap[j2 * 128:(j2 + 1) * 128, :]
                                .rearrange("p (c m) -> p c m", c=NK))
            bcol = sb_ws.tile([128, 1], f32, tag="bmj")
            nc.scalar.dma_start(bcol[:], bmap_ap[j2 * 128:(j2 + 1) * 128, :].bitcast(f32))
            for rh in range(RH):
                sl = slice(rh * 512, (rh + 1) * 512)
                ps_f = ps_o.tile([128, 512], f32, tag="ps_o")
                for c in range(NK):
                    nc.tensor.matmul(ps_f[:], wmj[:, c], o2T[:, c, sl],
                                     start=(c == 0), stop=(c == NK - 1))
                outsb = sb_ws.tile([128, 512], f32, tag="outsb")
                nc.vector.tensor_scalar(outsb[:], ps_f[:], bcol[:], None, Alu.add)
                nc.sync.dma_start(
                    out_ap[j2 * 128:(j2 + 1) * 128, sl], outsb[:])

    nc.compile()
    return nc


def _get_nc():
    global _CACHED_NC
    if _CACHED_NC is None:
        _CACHED_NC = _build_nc()
    return _CACHED_NC


def _shard_inputs(inputs):
    """Build the 8 per-core input maps (pure layout work, no arithmetic)."""
    x = np.ascontiguousarray(inputs["ctx_features"], dtype=np.float32)
    q1 = np.ascontiguousarray(inputs["sub_q1_features"], dtype=np.float32)
    q2 = np.ascontiguousarray(inputs["sub_q2_features"], dtype=np.float32)
    k1 = np.ascontiguousarray(inputs["sub_q1_attn_mask"], dtype=np.int32)
    k2 = np.ascontiguousarray(inputs["sub_q2_attn_mask"], dtype=np.int32)

    def wblocks(w_out):
        # w_out [D, 4D] -> wb = w_out.T [4D, D]; W_k = wb[kD:(k+1)D]
        wb = np.ascontiguousarray(w_out.T, dtype=np.float32)
        W1, W2, W3, W4 = (wb[k * D:(k + 1) * D] for k in range(4))

        def jmaj(W):  # j-major tiling for the lhsT stream
            return np.ascontiguousarray(
                W.reshape(NK, 128, NK, 128).transpose(2, 1, 0, 3).reshape(D, D))
        return jmaj(W1), np.ascontiguousarray(W2), jmaj(W3), np.ascontiguousarray(W4)

    w1t1, w2c1, w3t1, w4c1 = wblocks(inputs["w_out1"])
    w1t2, w2c2, w3t2, w4c2 = wblocks(inputs["w_out2"])

    wmT = inputs["w_map"].T.astype(np.float32)  # [D, 2D]
    wmt = np.ascontiguousarray(
        wmT.reshape(NK, 128, 16, 128).transpose(2, 1, 0, 3).reshape(D2, D))
    bmap = np.ascontiguousarray(
        np.asarray(inputs["b_map"], dtype=np.float32).reshape(D2, 1))

    def ptile_vec(*cols):  # [D] vectors -> [128, NK*k] p-major
        v = np.stack([np.asarray(c, dtype=np.float32) for c in cols], axis=-1)
        k = v.shape[-1]
        return np.ascontiguousarray(
            v.reshape(NK, 128, k).transpose(1, 0, 2).reshape(128, NK * k))

    stage_common = {
        "vec1": ptile_vec(inputs["w_in1"], inputs["w_mem1"], inputs["scale1"]),
        "vec2": ptile_vec(inputs["w_in2"], inputs["w_mem2"], inputs["scale2"]),
        "w1t1": w1t1, "w3t1": w3t1, "w2c1": w2c1, "w4c1": w4c1,
        "w1t2": w1t2, "w3t2": w3t2, "w2c2": w2c2, "w4c2": w4c2,
        "wmt": wmt, "bmap": bmap,
    }

    in_maps = []
    for core in range(N_CORES):
        b, h = divmod(core, 2)
        xT = x[b, h * R:(h + 1) * R, :].T  # [D, R]
        xt_tile = np.ascontiguousarray(
            xT.reshape(NK, 128, R).transpose(1, 0, 2).reshape(128, NK * R))
        m = {}
        for s, q, kk in ((1, q1, k1), (2, q2, k2)):
            mT = q[b].T  # [D, Q]
            m[f"m{s}t"] = np.ascontiguousarray(
                mT.reshape(NK, 128, Q_LEN).transpose(1, 0, 2).reshape(128, NK * Q_LEN))
            m[f"m{s}n"] = np.ascontiguousarray(q[b])
            m[f"mask{s}"] = np.ascontiguousarray(kk[b].reshape(Q_LEN, 1))
        in_maps.append({"xt": xt_tile, **m, **stage_common})
    return in_maps


def _gather_outputs(results):
    out = np.empty((B, C_LEN, D2), dtype=np.float32)
    for core in range(N_CORES):
        b, h = divmod(core, 2)
        out[b, h * R:(h + 1) * R, :] = results[core]["out"].T
    return out


def kernel(**inputs):
    nc = _get_nc()
    in_maps = _shard_inputs(inputs)
    last_err = None
    for _attempt in range(3):
        try:
            res = run_bass_kernel_spmd(nc, in_maps, core_ids=list(range(N_CORES)))
            return _gather_outputs(res.results)
        except Exception as e:  # transient device errors: retry
            last_err = e
    raise last_err



# revision 10
# speedup vs baseline: 1.4687x; 1.4687x over previous
"""Trainium2 Bass kernel for nn_NewModel_42356967473589 (dense_transformer).

Model: two BiAttention blocks + final linear mapping.
  o = BiAttn(ctx, q1) ; o = BiAttn(o, q2) ; out = o @ w_map.T + b_map

Sharding: 8 cores = (batch b in 0..3) x (context half h in 0..1).
Each core owns 1024 context rows of one batch. All compute is row-local
except the softmax-over-context (weight_two); its (sum-exp, weighted-sum)
stats are combined across the pair of cores sharing a batch via a tiny
pairwise AllReduce, overlapped with the large matmuls.

Math restructure (per stage, X = stage input [C,D], M = memory [Q,D]):
  out = X@W1 + o1@W2 + (X*o1)@W3 + (t*o1)@W4      (W_k = w_out[:, kD:(k+1)D].T)
  o1 = P@M (rank Q=64), t broadcast over rows =>
  o1@W2 + (t*o1)@W4 = P @ (M @ (W2 + t*W4))        (rank-64 path)

v2: all heavy matmul operands in bf16 (psum stays fp32).  The softmax
reciprocal runs on a [128,4] column layout (DVE reciprocal is 8 cyc/elem
per partition lane, so a [1,512] row costs 4.3us while [128,4] costs
~0.2us); the row broadcast back is a stride-0-lhsT identity matmul.
Engine queues are ordered so the PE never idles >3.4us (HAM clock gate).
"""

import numpy as np
import ml_dtypes

import concourse.bacc as bacc
import concourse.tile as tile
from concourse import mybir
from concourse.bass_utils import run_bass_kernel_spmd
from contextlib import ExitStack
import bass_rust

f32 = mybir.dt.float32
f32r = mybir.dt.float32r
bf16 = mybir.dt.bfloat16
i32 = mybir.dt.int32
Alu = mybir.AluOpType
AF = bass_rust.ActivationFunctionType
AX = bass_rust.AxisListType
RedOp = bass_rust.ReduceOp

B, C_LEN, Q_LEN, D = 4, 2048, 64, 1024
N_CORES = 8
R = C_LEN // 2          # rows per core
NK = D // 128           # contraction chunks
RH = R // 512           # row halves (moving-dim tiles)
D2 = 2 * D
NEGBIG = 10000.0

_CACHED_NC = None


def _build_nc():
    nc = bacc.Bacc("TRN2", target_bir_lowering=False, debug=False,
                   num_devices=N_CORES)

    # ---- per-core DRAM I/O (host pre-tiled layouts, see _shard_inputs) ----
    xt_ap = nc.dram_tensor("xt", [128, NK * R], bf16, kind="ExternalInput").ap()
    m_t = [nc.dram_tensor(f"m{s}t", [128, NK * Q_LEN], bf16, kind="ExternalInput").ap() for s in (1, 2)]
    m_n = [nc.dram_tensor(f"m{s}n", [Q_LEN, D], bf16, kind="ExternalInput").ap() for s in (1, 2)]
    vecb = [nc.dram_tensor(f"vecb{s}", [128, NK * 2], bf16, kind="ExternalInput").ap() for s in (1, 2)]
    sclf = [nc.dram_tensor(f"sclf{s}", [128, NK], f32, kind="ExternalInput").ap() for s in (1, 2)]
    msk = [nc.dram_tensor(f"mask{s}", [Q_LEN, 1], i32, kind="ExternalInput").ap() for s in (1, 2)]
    w13 = [nc.dram_tensor(f"w13_{s}", [128, NK * 2 * NK * 128], bf16, kind="ExternalInput").ap() for s in (1, 2)]
    w24 = [nc.dram_tensor(f"w24_{s}", [128, NK * 2 * D], bf16, kind="ExternalInput").ap() for s in (1, 2)]
    wmt_ap = nc.dram_tensor("wmt", [128, 16 * NK * 128], bf16, kind="ExternalInput").ap()
    bmap_ap = nc.dram_tensor("bmap", [128, 16], f32, kind="ExternalInput").ap()
    ident_ap = nc.dram_tensor("ident", [128, 128], f32r, kind="ExternalInput").ap()
    out_ap = nc.dram_tensor("out", [D2, R], f32, kind="ExternalOutput").ap()

    with tile.TileContext(nc) as tc, ExitStack() as ctx:
        sb_x = ctx.enter_context(tc.tile_pool(name="sb_x", bufs=2))
        sb_xo = ctx.enter_context(tc.tile_pool(name="sb_xo", bufs=1))
        sb_w13 = ctx.enter_context(tc.tile_pool(name="sb_w13", bufs=16))
        sb_w24 = ctx.enter_context(tc.tile_pool(name="sb_w24", bufs=8))
        sb_wm = ctx.enter_context(tc.tile_pool(name="sb_wm", bufs=8))
        sb_ws = ctx.enter_context(tc.tile_pool(name="sb_ws", bufs=2))
        sb_st = ctx.enter_context(tc.tile_pool(name="sb_st", bufs=1))
        sb_rh = ctx.enter_context(tc.tile_pool(name="sb_rh", bufs=2))
        ps_att = ctx.enter_context(tc.tile_pool(name="ps_att", bufs=2, space="PSUM"))
        ps_big = ctx.enter_context(tc.tile_pool(name="ps_big", bufs=3, space="PSUM"))
        ps_sm = ctx.enter_context(tc.tile_pool(name="ps_sm", bufs=1, space="PSUM"))
        ps_bc = ctx.enter_context(tc.tile_pool(name="ps_bc", bufs=2, space="PSUM"))
        dram = ctx.enter_context(tc.tile_pool(name="dram", bufs=2, space="DRAM"))

        # ---- constants ----
        ident = sb_st.tile([128, 128], f32r, tag="ident")
        nc.sync.dma_start(ident[:], ident_ap[:])
        ones_row = sb_st.tile([1, 128], f32r, tag="ones_row")
        nc.vector.memset(ones_row[:].bitcast(f32), 1.0)
        ones_qb = sb_st.tile([Q_LEN, 1], f32, tag="ones_qb")
        nc.vector.memset(ones_qb[:], 1.0)
        ones_qb16 = sb_st.tile([Q_LEN, 1], bf16, tag="ones_qb16")
        nc.vector.tensor_copy(ones_qb16[:], ones_qb[:])

        # ---- stage input 1: X^T (bf16, one big DMA on gpsimd queue) ----
        xt0 = sb_x.tile([128, NK, R], bf16, tag="xt")
        nc.gpsimd.dma_start(xt0[:], xt_ap[:].rearrange("p (c r) -> p c r", c=NK))

        # stage-1 weights: w13 j-tiles on sync queue, w24 chunks on gpsimd
        w13_t = {1: [], 2: []}
        w24_t = {1: [], 2: []}

        def load_w13(s):
            for j in range(NK):
                w13j = sb_w13.tile([128, 2, NK, 128], bf16, tag="w13")
                nc.sync.dma_start(
                    w13j[:], w13[s - 1][:, j * 2048:(j + 1) * 2048]
                    .rearrange("p (t c m) -> p t c m", t=2, c=NK))
                w13_t[s].append(w13j)

        def load_w24(s):
            for c in range(NK):
                w24c = sb_w24.tile([128, 2, D], bf16, tag="w24")
                nc.gpsimd.dma_start(
                    w24c[:], w24[s - 1][:, c * 2 * D:(c + 1) * 2 * D]
                    .rearrange("p (t m) -> p t m", t=2))
                w24_t[s].append(w24c)

        def load_stage_consts(s):
            vb = sb_st.tile([128, NK, 2], bf16, tag=f"vb{s}")
            nc.sync.dma_start(vb[:], vecb[s - 1][:].rearrange("p (c k) -> p c k", c=NK))
            sf = sb_st.tile([128, NK], f32, tag=f"sf{s}")
            nc.sync.dma_start(sf[:], sclf[s - 1][:])
            mT = sb_st.tile([128, NK, Q_LEN], bf16, tag=f"mT{s}")
            nc.sync.dma_start(mT[:], m_t[s - 1][:].rearrange("p (c q) -> p c q", c=NK))
            mN = sb_st.tile([Q_LEN, D], bf16, tag=f"mN{s}")
            nc.sync.dma_start(mN[:], m_n[s - 1][:])
            mask_i = sb_st.tile([Q_LEN, 1], i32, tag=f"mask_i{s}")
            nc.sync.dma_start(mask_i[:], msk[s - 1][:])
            return vb, sf, mT, mN, mask_i

        load_stage_consts_cache = {}
        load_stage_consts_cache[1] = load_stage_consts(1)
        load_w13(1)
        load_w24(1)

        def run_stage(s, Xt):
            """One BiAttention stage; returns o^T tile [128, NK, R] bf16."""
            sfx = f"_s{s}"
            vb, sf, mT, mN, mask_i = load_stage_consts_cache[s]

            # ---------- prep: mst = [M^T * scale | w_in], memory_dot, mbias --
            mst = sb_st.tile([128, NK, Q_LEN + 1], bf16, tag="mst" + sfx)
            nc.vector.tensor_copy(mst[:, :, 0:Q_LEN], mT[:])
            nc.vector.tensor_copy(mst[:, :, Q_LEN:Q_LEN + 1], vb[:, :, 0:1])
            for c in range(NK):
                nc.vector.tensor_scalar(mst[:, c, 0:Q_LEN], mst[:, c, 0:Q_LEN],
                                        sf[:, c:c + 1], None, Alu.mult)

            # memory_dot = M @ w_mem  -> psum [Q,1]
            ps_md = ps_sm.tile([128, 4], f32, tag="ps_sm")
            for c in range(NK):
                nc.tensor.matmul(ps_md[0:Q_LEN, 0:1], mT[:, c], vb[:, c, 1:2],
                                 start=(c == 0), stop=(c == NK - 1))
            maskf = sb_st.tile([Q_LEN, 1], f32, tag="maskf" + sfx)
            nc.vector.tensor_copy(maskf[:], mask_i[:])
            mbias = sb_st.tile([Q_LEN, 1], f32, tag="mbias" + sfx)
            nc.vector.tensor_scalar(mbias[:], maskf[:], NEGBIG, -NEGBIG, Alu.mult, Alu.add)
            nc.vector.tensor_tensor(mbias[:], mbias[:], ps_md[0:Q_LEN, 0:1], Alu.add)

            # ---------- per-rh tiles ----------
            P = sb_st.tile([Q_LEN, R], bf16, tag="P" + sfx)
            Es = []
            eids = []
            ps_scs = []
            # scores for both rh first (PE front-loading)
            for rh in range(RH):
                sl = slice(rh * 512, (rh + 1) * 512)
                ps_sc = ps_att.tile([Q_LEN + 1, 512], f32, tag="ps_att")
                for c in range(NK):
                    nc.tensor.matmul(ps_sc[:], mst[:, c], Xt[:, c, sl],
                                     start=(c == 0), stop=(c == NK - 1))
                ps_scs.append(ps_sc)
                E = sb_rh.tile([Q_LEN, 512], bf16, tag="E")
                nc.scalar.activation(E[:], ps_sc[0:Q_LEN], AF.Exp,
                                     bias=mbias[:], scale=1.0)
                eid = sb_rh.tile([1, 512], f32, tag="eid")
                nc.scalar.activation(eid[:], ps_sc[Q_LEN:Q_LEN + 1], AF.Exp)
                Es.append(E)
                eids.append(eid)

            # column softmax sums in [128,4] layout, reciprocal, broadcast back
            rbs = []
            for rh in range(RH):
                E = Es[rh]
                ps_l1c = ps_sm.tile([128, 4], f32, tag="ps_sm")
                for q4 in range(4):
                    nc.tensor.matmul(ps_l1c[:, q4:q4 + 1],
                                     E[:, q4 * 128:(q4 + 1) * 128], ones_qb16[:],
                                     start=True, stop=True)
                l1r = sb_rh.tile([128, 4], f32r, tag="l1r")
                with nc.allow_low_precision(reason="softmax scale in f32r"):
                    nc.vector.reciprocal(l1r[:], ps_l1c[:])
                ps_rb = ps_bc.tile([128, 512], f32, tag="ps_bc")
                for q4 in range(4):
                    nc.tensor.matmul(
                        ps_rb[0:Q_LEN, q4 * 128:(q4 + 1) * 128],
                        l1r[:, q4:q4 + 1].broadcast_to([128, Q_LEN]),
                        ident[:], start=True, stop=True)
                rbs.append(ps_rb)
                nc.vector.tensor_tensor(P[:, rh * 512:(rh + 1) * 512],
                                        E[:], ps_rb[0:Q_LEN], Alu.mult)

            # o1^T chunks + XO = Xt * o1
            XO = sb_xo.tile([128, NK, R], bf16, tag="xo")
            for rh in range(RH):
                sl = slice(rh * 512, (rh + 1) * 512)
                for c in range(NK):
                    ps_o1 = ps_att.tile([128, 512], f32, tag="ps_att")
                    nc.tensor.matmul(ps_o1[:], mN[:, c * 128:(c + 1) * 128], P[:, sl],
                                     start=True, stop=True)
                    nc.vector.tensor_tensor(XO[:, c, sl],
                                            Xt[:, c, sl], ps_o1[:], Alu.mult)

            # gpsimd max over q (for weight_two) — issue early, runs async
            mxs = []
            for rh in range(RH):
                mx = sb_rh.tile([Q_LEN, 512], f32, tag="mx")
                nc.gpsimd.partition_all_reduce(mx[:], Es[rh][:], Q_LEN, RedOp.max)
                mxs.append(mx)

            # ---------- big blocks: oT = W1^T X + W3^T XO ----------
            oT = sb_x.tile([128, NK, R], bf16, tag="xt")
            w13s = w13_t[s]

            def big_group(j, rh):
                sl = slice(rh * 512, (rh + 1) * 512)
                ps_ab = ps_big.tile([128, 512], f32, tag="ps_big")
                for c in range(NK):
                    nc.tensor.matmul(ps_ab[:], w13s[j][:, 0, c], Xt[:, c, sl],
                                     start=(c == 0), stop=False)
                for c in range(NK):
                    nc.tensor.matmul(ps_ab[:], w13s[j][:, 1, c], XO[:, c, sl],
                                     start=False, stop=(c == NK - 1))
                nc.scalar.activation(oT[:, j, sl], ps_ab[:], AF.Copy)

            # interleave the first groups so the PE never waits on XO(rh=1)
            big_group(0, 0)
            big_group(1, 0)
            big_group(0, 1)
            big_group(1, 1)

            # ---------- weight-two stats (PE is busy on big blocks) ----------
            vh = sb_st.tile([128, 2 * NK], f32, tag="vh" + sfx)
            l2col = sb_st.tile([1, 2], f32, tag="l2col" + sfx)

            def stats_rh(rh):
                e2 = sb_rh.tile([1, 512], f32r, tag="e2")
                nc.vector.tensor_tensor(e2[:], mxs[rh][0:1],
                                        eids[rh][:], Alu.mult)
                nc.vector.reduce_sum(l2col[:, rh:rh + 1], e2[:].bitcast(f32), AX.X)
                ps_eb = ps_bc.tile([128, 512], f32, tag="ps_bc")
                nc.tensor.matmul(ps_eb[:], ones_row[:], e2[:], start=True, stop=True)
                e2b = sb_rh.tile([128, 512], bf16, tag="e2b")
                nc.scalar.activation(e2b[:], ps_eb[:], AF.Copy)
                scrv = sb_rh.tile([128, 512], bf16, tag="scrv")
                sl = slice(rh * 512, (rh + 1) * 512)
                for c in range(NK):
                    nc.vector.scalar_tensor_tensor(
                        scrv[:], Xt[:, c, sl], 1.0, e2b[:],
                        Alu.mult, Alu.mult,
                        accum_out=vh[:, 2 * c + rh:2 * c + rh + 1])

            for rh in range(RH):
                stats_rh(rh)

            l2 = sb_st.tile([1, 1], f32, tag="l2" + sfx)
            nc.vector.reduce_sum(l2[:], l2col[:], AX.X)
            vsum = sb_st.tile([128, NK], f32, tag="vsum" + sfx)
            vh3 = vh[:].rearrange("p (c t) -> p c t", t=2)
            nc.vector.tensor_tensor(vsum[:], vh3[:, :, 0], vh3[:, :, 1], Alu.add)

            # ---------- pairwise AllReduce of (v, l2) ----------
            colsb = sb_st.tile([128, 16], f32, tag="colsb" + sfx)
            nc.vector.memset(colsb[:], 0.0)
            nc.vector.tensor_copy(colsb[:, 0:NK], vsum[:])
            nc.vector.tensor_copy(colsb[0:1, NK:NK + 1], l2[:])
            nc.vector.tensor_copy(colsb[0:1, NK + 1:NK + 2], l2[:])
            cin = dram.tile([128, 16], f32, tag="cin" + sfx)
            cout = dram.tile([128, 16], f32, tag="cout" + sfx)
            nc.sync.dma_start(cin[:], colsb[:])
            nc.gpsimd.collective_compute(
                "AllReduce", Alu.add,
                replica_groups=[[0, 1], [2, 3], [4, 5], [6, 7]],
                ins=[cin[:].opt()], outs=[cout[:].opt()])
            colg = sb_st.tile([128, 16], f32, tag="colg" + sfx)
            nc.sync.dma_start(colg[:], cout[:])

            # ---------- remaining big blocks ----------
            for j in range(2, NK):
                for rh in range(RH):
                    big_group(j, rh)

            # prefetch next-stage / final weights while the PE crunches
            if s == 1:
                load_stage_consts_cache[2] = load_stage_consts(2)
                load_w13(2)
            else:
                for j2 in range(16):
                    wmj = sb_wm.tile([128, NK, 128], bf16, tag="wm")
                    nc.sync.dma_start(
                        wmj[:], wmt_ap[:, j2 * 1024:(j2 + 1) * 1024]
                        .rearrange("p (c m) -> p c m", c=NK))
                    wm_tiles.append(wmj)
                nc.sync.dma_start(bcol_all[:], bmap_ap[:])

            # ---------- collective-dependent tail ----------
            linv = sb_st.tile([1, 2], f32r, tag="linv" + sfx)
            with nc.allow_low_precision(reason="weight-two scale in f32r"):
                nc.vector.reciprocal(linv[:], colg[0:1, NK:NK + 2])
            ps_tb = ps_sm.tile([128, 4], f32, tag="ps_sm")
            nc.tensor.matmul(ps_tb[:, 0:2], ones_row[:], linv[:], start=True, stop=True)
            tvec = sb_st.tile([128, NK], f32, tag="tvec" + sfx)
            nc.vector.tensor_scalar(tvec[:], colg[:, 0:NK], ps_tb[:, 0:1], None, Alu.mult)
            w24s = w24_t[s]
            w4sc = []
            for c in range(NK):
                w4c = sb_ws.tile([128, D], bf16, tag="w4sc")
                nc.vector.tensor_scalar(w4c[:], w24s[c][:, 1], tvec[:, c:c + 1],
                                        None, Alu.mult)
                w4sc.append(w4c)

            # R = M^T (W2 + t*W4)  [two psum halves], then Rsb bf16
            ps_r = []
            for hf in range(2):
                ps_ri = ps_big.tile([128, 512], f32, tag="ps_big")
                ps_r.append(ps_ri)
                slh = slice(hf * 512, (hf + 1) * 512)
                for c in range(NK):
                    nc.tensor.matmul(ps_ri[0:Q_LEN], mT[:, c], w24s[c][:, 0, slh],
                                     start=(c == 0), stop=False)
                for c in range(NK):
                    nc.tensor.matmul(ps_ri[0:Q_LEN], mT[:, c], w4sc[c][:, slh],
                                     start=False, stop=(c == NK - 1))
            Rsb = sb_st.tile([Q_LEN, D], bf16, tag="Rsb" + sfx)
            for hf in range(2):
                nc.scalar.activation(Rsb[:, hf * 512:(hf + 1) * 512],
                                     ps_r[hf][0:Q_LEN], AF.Copy)

            # rank-64 correction (rh-major so the next stage starts sooner)
            for rh in range(RH):
                sl = slice(rh * 512, (rh + 1) * 512)
                for j in range(NK):
                    ps_c = ps_att.tile([128, 512], f32, tag="ps_att")
                    nc.tensor.matmul(ps_c[:], Rsb[:, j * 128:(j + 1) * 128], P[:, sl],
                                     start=True, stop=True)
                    nc.vector.tensor_tensor(oT[:, j, sl],
                                            oT[:, j, sl], ps_c[:], Alu.add)
            return oT

        wm_tiles = []
        bcol_all = sb_st.tile([128, 16], f32, tag="bcol_all")

        o1T = run_stage(1, xt0)
        load_w24(2)
        o2T = run_stage(2, o1T)

        # ---------- final linear (transposed): outT = w_mapT.T @ o2T + b ----
        for j2 in range(16):
            wmj = wm_tiles[j2]
            for rh in range(RH):
                sl = slice(rh * 512, (rh + 1) * 512)
                ps_f = ps_big.tile([128, 512], f32, tag="ps_big")
                for c in range(NK):
                    nc.tensor.matmul(ps_f[:], wmj[:, c], o2T[:, c, sl],
                                     start=(c == 0), stop=(c == NK - 1))
                outsb = sb_ws.tile([128, 512], f32, tag="outsb")
                if (j2 + rh) % 2 == 0:
                    nc.scalar.activation(outsb[:], ps_f[:], AF.Identity,
                                         bias=bcol_all[:, j2:j2 + 1], scale=1.0)
                else:
                    nc.vector.tensor_scalar(outsb[:], ps_f[:],
                                            bcol_all[:, j2:j2 + 1], None, Alu.add)
                nc.sync.dma_start(out_ap[j2 * 128:(j2 + 1) * 128, sl], outsb[:])

    nc.compile()
    return nc


def _get_nc():
    global _CACHED_NC
    if _CACHED_NC is None:
        _CACHED_NC = _build_nc()
    return _CACHED_NC


def _bf(a):
    return np.ascontiguousarray(np.asarray(a, dtype=np.float32).astype(ml_dtypes.bfloat16))


def _shard_inputs(inputs):
    """Build the 8 per-core input maps (pure layout work, no arithmetic)."""
    x = np.asarray(inputs["ctx_features"], dtype=np.float32)
    q1 = np.asarray(inputs["sub_q1_features"], dtype=np.float32)
    q2 = np.asarray(inputs["sub_q2_features"], dtype=np.float32)
    k1 = np.ascontiguousarray(np.asarray(inputs["sub_q1_attn_mask"], dtype=np.int32))
    k2 = np.ascontiguousarray(np.asarray(inputs["sub_q2_attn_mask"], dtype=np.int32))

    def wpack13(w_out):
        # w_out [D, 4D] -> wb = w_out.T [4D, D]; W_k = wb[kD:(k+1)D]
        wb = np.asarray(w_out, dtype=np.float32).T
        W1, W3 = wb[0:D], wb[2 * D:3 * D]
        # pack[p, j, t, c, m] = Wt[c*128+p, j*128+m]
        def v(W):  # [D_in, D_out] -> [c,p,j,m]
            return W.reshape(NK, 128, NK, 128)
        pk = np.stack([v(W1), v(W3)], axis=0)  # [t, c, p, j, m]
        pk = pk.transpose(2, 3, 0, 1, 4)       # [p, j, t, c, m]
        return _bf(pk.reshape(128, NK * 2 * NK * 128))

    def wpack24(w_out):
        wb = np.asarray(w_out, dtype=np.float32).T
        W2, W4 = wb[D:2 * D], wb[3 * D:4 * D]
        # pack[p, c, t, m] = Wt[c*128+p, m]
        pk = np.stack([W2.reshape(NK, 128, D), W4.reshape(NK, 128, D)], axis=0)
        pk = pk.transpose(2, 1, 0, 3)          # [p, c, t, m]
        return _bf(pk.reshape(128, NK * 2 * D))

    w13_1 = wpack13(inputs["w_out1"])
    w13_2 = wpack13(inputs["w_out2"])
    w24_1 = wpack24(inputs["w_out1"])
    w24_2 = wpack24(inputs["w_out2"])

    wmT = np.asarray(inputs["w_map"], dtype=np.float32).T  # [D, 2D]
    # wmt[p, j2, c, m] = wmT[c*128+p, j2*128+m]
    wmt = wmT.reshape(NK, 128, 16, 128).transpose(1, 2, 0, 3)
    wmt = _bf(wmt.reshape(128, 16 * NK * 128))
    bmap = np.ascontiguousarray(
        np.asarray(inputs["b_map"], dtype=np.float32).reshape(16, 128).T)

    def ptile(vec_list, dtype):  # [D] vectors -> [128, NK*k] p-major
        v = np.stack([np.asarray(c, dtype=np.float32) for c in vec_list], axis=-1)
        k = v.shape[-1]
        out = v.reshape(NK, 128, k).transpose(1, 0, 2).reshape(128, NK * k)
        if dtype == "bf16":
            return _bf(out)
        return np.ascontiguousarray(out)

    ident = np.ascontiguousarray(np.eye(128, dtype=np.float32))

    stage_common = {
        "vecb1": ptile([inputs["w_in1"], inputs["w_mem1"]], "bf16"),
        "vecb2": ptile([inputs["w_in2"], inputs["w_mem2"]], "bf16"),
        "sclf1": ptile([inputs["scale1"]], "f32"),
        "sclf2": ptile([inputs["scale2"]], "f32"),
        "w13_1": w13_1, "w13_2": w13_2, "w24_1": w24_1, "w24_2": w24_2,
        "wmt": wmt, "bmap": bmap, "ident": ident,
    }

    in_maps = []
    for core in range(N_CORES):
        b, h = divmod(core, 2)
        xT = x[b, h * R:(h + 1) * R, :].T  # [D, R]
        xt_tile = _bf(xT.reshape(NK, 128, R).transpose(1, 0, 2).reshape(128, NK * R))
        m = {}
        for s, q, kk in ((1, q1, k1), (2, q2, k2)):
            mT = q[b].T  # [D, Q]
            m[f"m{s}t"] = _bf(
                mT.reshape(NK, 128, Q_LEN).transpose(1, 0, 2).reshape(128, NK * Q_LEN))
            m[f"m{s}n"] = _bf(q[b])
            m[f"mask{s}"] = np.ascontiguousarray(kk[b].reshape(Q_LEN, 1))
        in_maps.append({"xt": xt_tile, **m, **stage_common})
    return in_maps


def _gather_outputs(results):
    out = np.empty((B, C_LEN, D2), dtype=np.float32)
    for core in range(N_CORES):
        b, h = divmod(core, 2)
        out[b, h * R:(h + 1) * R, :] = results[core]["out"].T
    return out


def kernel(**inputs):
    nc = _get_nc()
    in_maps = _shard_inputs(inputs)
    last_err = None
    for _attempt in range(3):
        try:
            res = run_bass_kernel_spmd(nc, in_maps, core_ids=list(range(N_CORES)))
            return _gather_outputs(res.results)
        except Exception as e:  # transient device errors: retry
            last_err = e
    raise last_err


# revision 12
# speedup vs baseline: 1.6277x; 1.1083x over previous
"""Trainium2 Bass kernel for nn_NewModel_42356967473589 (dense_transformer).

Model: two BiAttention blocks + final linear mapping.
  o = BiAttn(ctx, q1) ; o = BiAttn(o, q2) ; out = o @ w_map.T + b_map

Sharding: 8 cores = (batch b in 0..3) x (context half h in 0..1).
Each core owns 1024 context rows of one batch. All compute is row-local
except the softmax-over-context (weight_two); its (sum-exp, weighted-sum)
stats are combined across the pair of cores sharing a batch via a tiny
pairwise AllReduce, overlapped with the large matmuls.

Math restructure (per stage, X = stage input [C,D], M = memory [Q,D]):
  out = X@W1 + o1@W2 + (X*o1)@W3 + (t*o1)@W4      (W_k = w_out[:, kD:(k+1)D].T)
  o1 = P@M (rank Q=64), t broadcast over rows =>
  o1@W2 + (t*o1)@W4 = P @ (M @ (W2 + t*W4))        (rank-64 path)

v3: all heavy matmul operands bf16 (psum fp32); softmax reciprocal in a
[128,4] column layout (DVE reciprocal is 8 cyc/elem/lane, so [1,512] on
one partition costs 4.3us vs ~0.2us here), row-broadcast back via a
stride-0-lhsT identity matmul.  PE queue is kept dense end-to-end (HAM
clock gate re-throttles after idle): o1 matmuls interleave with the
W1-parts of the first output groups, the rank-64 correction of the last
two j-blocks rides inside their psum accumulation groups, and weight
DMAs are dependency-gated on the gpsimd queue so the stage-1 input
transfer gets full HBM bandwidth at startup.
"""

import numpy as np
import ml_dtypes

import concourse.bacc as bacc
import concourse.tile as tile
from concourse import mybir
from concourse.bass_utils import run_bass_kernel_spmd
from contextlib import ExitStack
import bass_rust

f32 = mybir.dt.float32
f32r = mybir.dt.float32r
bf16 = mybir.dt.bfloat16
i32 = mybir.dt.int32
Alu = mybir.AluOpType
AF = bass_rust.ActivationFunctionType
AX = bass_rust.AxisListType
RedOp = bass_rust.ReduceOp

B, C_LEN, Q_LEN, D = 4, 2048, 64, 1024
N_CORES = 8
R = C_LEN // 2          # rows per core
NK = D // 128           # contraction chunks
RH = R // 512           # row halves (moving-dim tiles)
D2 = 2 * D
NEGBIG = 10000.0

_CACHED_NC = None


def _build_nc():
    nc = bacc.Bacc("TRN2", target_bir_lowering=False, debug=False,
                   num_devices=N_CORES)

    # ---- per-core DRAM I/O (host pre-tiled layouts, see _shard_inputs) ----
    xt_ap = nc.dram_tensor("xt", [128, NK * R], bf16, kind="ExternalInput").ap()
    m_t = [nc.dram_tensor(f"m{s}t", [128, NK * Q_LEN], bf16, kind="ExternalInput").ap() for s in (1, 2)]
    m_n = [nc.dram_tensor(f"m{s}n", [Q_LEN, D], bf16, kind="ExternalInput").ap() for s in (1, 2)]
    vecb = [nc.dram_tensor(f"vecb{s}", [128, NK * 2], bf16, kind="ExternalInput").ap() for s in (1, 2)]
    sclf = [nc.dram_tensor(f"sclf{s}", [128, NK], f32, kind="ExternalInput").ap() for s in (1, 2)]
    msk = [nc.dram_tensor(f"mask{s}", [Q_LEN, 1], i32, kind="ExternalInput").ap() for s in (1, 2)]
    w13 = [nc.dram_tensor(f"w13_{s}", [128, NK * 2 * NK * 128], bf16, kind="ExternalInput").ap() for s in (1, 2)]
    w24 = [nc.dram_tensor(f"w24_{s}", [128, NK * 2 * D], bf16, kind="ExternalInput").ap() for s in (1, 2)]
    wmt_ap = nc.dram_tensor("wmt", [128, 16 * NK * 128], bf16, kind="ExternalInput").ap()
    bmap_ap = nc.dram_tensor("bmap", [128, 16], f32, kind="ExternalInput").ap()
    ident_ap = nc.dram_tensor("ident", [128, 128], f32r, kind="ExternalInput").ap()
    out_ap = nc.dram_tensor("out", [D2, R], bf16, kind="ExternalOutput").ap()

    with tile.TileContext(nc) as tc, ExitStack() as ctx:
        sb_x = ctx.enter_context(tc.tile_pool(name="sb_x", bufs=2))
        sb_xo = ctx.enter_context(tc.tile_pool(name="sb_xo", bufs=1))
        sb_w13 = ctx.enter_context(tc.tile_pool(name="sb_w13", bufs=16))
        sb_w24 = ctx.enter_context(tc.tile_pool(name="sb_w24", bufs=8))
        sb_wm = ctx.enter_context(tc.tile_pool(name="sb_wm", bufs=8))
        sb_ws = ctx.enter_context(tc.tile_pool(name="sb_ws", bufs=2))
        sb_st = ctx.enter_context(tc.tile_pool(name="sb_st", bufs=1))
        sb_rh = ctx.enter_context(tc.tile_pool(name="sb_rh", bufs=2))
        ps_att = ctx.enter_context(tc.tile_pool(name="ps_att", bufs=2, space="PSUM"))
        ps_big = ctx.enter_context(tc.tile_pool(name="ps_big", bufs=3, space="PSUM"))
        ps_sm = ctx.enter_context(tc.tile_pool(name="ps_sm", bufs=1, space="PSUM"))
        ps_bc = ctx.enter_context(tc.tile_pool(name="ps_bc", bufs=2, space="PSUM"))
        dram = ctx.enter_context(tc.tile_pool(name="dram", bufs=2, space="DRAM"))

        # ---- constants ----
        ones_row = sb_st.tile([1, 128], f32r, tag="ones_row")
        nc.vector.memset(ones_row[:].bitcast(f32), 1.0)
        ones_qb16 = sb_st.tile([Q_LEN, 1], bf16, tag="ones_qb16")
        nc.vector.memset(ones_qb16[:], 1.0)

        # ---- stage-1 input + const DMAs (sync queue: small stuff only) ----
        def load_stage_consts(s):
            vb = sb_st.tile([128, NK, 2], bf16, tag=f"vb{s}")
            nc.sync.dma_start(vb[:], vecb[s - 1][:].rearrange("p (c k) -> p c k", c=NK))
            sf = sb_st.tile([128, NK], f32, tag=f"sf{s}")
            nc.sync.dma_start(sf[:], sclf[s - 1][:])
            mT = sb_st.tile([128, NK, Q_LEN], bf16, tag=f"mT{s}")
            nc.sync.dma_start(mT[:], m_t[s - 1][:].rearrange("p (c q) -> p c q", c=NK))
            mN = sb_st.tile([Q_LEN, D], bf16, tag=f"mN{s}")
            nc.sync.dma_start(mN[:], m_n[s - 1][:])
            mask_i = sb_st.tile([Q_LEN, 1], i32, tag=f"mask_i{s}")
            nc.sync.dma_start(mask_i[:], msk[s - 1][:])
            return vb, sf, mT, mN, mask_i

        consts = {1: load_stage_consts(1)}
        ident = sb_st.tile([128, 128], f32r, tag="ident")
        nc.sync.dma_start(ident[:], ident_ap[:])
        consts[2] = load_stage_consts(2)

        # xt first on gpsimd so it gets HBM bandwidth; weights follow,
        # the late ones gated behind the (E-dependent) gpsimd max ops.
        xt0 = sb_x.tile([128, NK, R], bf16, tag="xt")
        nc.gpsimd.dma_start(xt0[:], xt_ap[:].rearrange("p (c r) -> p c r", c=NK))

        w13_t = {1: [], 2: []}
        w24_t = {1: [], 2: []}

        def load_w13(s, js):
            for j in js:
                w13j = sb_w13.tile([128, 2, NK, 128], bf16, tag="w13")
                nc.gpsimd.dma_start(
                    w13j[:], w13[s - 1][:, j * 2048:(j + 1) * 2048]
                    .rearrange("p (t c m) -> p t c m", t=2, c=NK))
                w13_t[s].append(w13j)

        def load_w24(s):
            for c in range(NK):
                w24c = sb_w24.tile([128, 2, D], bf16, tag="w24")
                nc.gpsimd.dma_start(
                    w24c[:], w24[s - 1][:, c * 2 * D:(c + 1) * 2 * D]
                    .rearrange("p (t m) -> p t m", t=2))
                w24_t[s].append(w24c)

        load_w13(1, range(0, 4))

        # ---- per-stage prep: mst, memory_dot, mbias (runs early) ----
        def prep_stage(s):
            vb, sf, mT, mN, mask_i = consts[s]
            sfx = f"_s{s}"
            mst = sb_st.tile([128, NK, Q_LEN + 1], bf16, tag="mst" + sfx)
            nc.vector.tensor_copy(mst[:, :, 0:Q_LEN], mT[:])
            nc.vector.tensor_copy(mst[:, :, Q_LEN:Q_LEN + 1], vb[:, :, 0:1])
            for c in range(NK):
                nc.vector.tensor_scalar(mst[:, c, 0:Q_LEN], mst[:, c, 0:Q_LEN],
                                        sf[:, c:c + 1], None, Alu.mult)
            ps_md = ps_sm.tile([128, 4], f32, tag="ps_sm")
            for c in range(NK):
                nc.tensor.matmul(ps_md[0:Q_LEN, 0:1], mT[:, c], vb[:, c, 1:2],
                                 start=(c == 0), stop=(c == NK - 1))
            maskf = sb_st.tile([Q_LEN, 1], f32, tag="maskf" + sfx)
            nc.vector.tensor_copy(maskf[:], mask_i[:])
            mbias = sb_st.tile([Q_LEN, 1], f32, tag="mbias" + sfx)
            nc.vector.tensor_scalar(mbias[:], maskf[:], NEGBIG, -NEGBIG, Alu.mult, Alu.add)
            nc.vector.tensor_tensor(mbias[:], mbias[:], ps_md[0:Q_LEN, 0:1], Alu.add)
            return mst, mbias

        prep = {1: prep_stage(1)}
        wm_tiles = []
        bcol_all = sb_st.tile([128, 16], f32, tag="bcol_all")

        def run_stage(s, Xt):
            """One BiAttention stage; returns o^T tile [128, NK, R] bf16."""
            sfx = f"_s{s}"
            vb, sf, mT, mN, mask_i = consts[s]
            mst, mbias = prep[s]

            # ---------- scores for both row-halves ----------
            Es, eids = [], []
            for rh in range(RH):
                sl = slice(rh * 512, (rh + 1) * 512)
                ps_sc = ps_att.tile([Q_LEN + 1, 512], f32, tag="ps_att")
                for c in range(NK):
                    nc.tensor.matmul(ps_sc[:], mst[:, c], Xt[:, c, sl],
                                     start=(c == 0), stop=(c == NK - 1))
                E = sb_rh.tile([Q_LEN, 512], bf16, tag="E")
                nc.scalar.activation(E[:], ps_sc[0:Q_LEN], AF.Exp,
                                     bias=mbias[:], scale=1.0)
                eid = sb_rh.tile([1, 512], f32, tag="eid")
                nc.scalar.activation(eid[:], ps_sc[Q_LEN:Q_LEN + 1], AF.Exp)
                Es.append(E)
                eids.append(eid)

            # gpsimd max over q (for weight_two) — also gates late w13 DMAs
            mxs = []
            for rh in range(RH):
                mx = sb_rh.tile([Q_LEN, 512], f32, tag="mx")
                nc.gpsimd.partition_all_reduce(mx[:], Es[rh][:], Q_LEN, RedOp.max)
                mxs.append(mx)
            if s == 1:
                load_w13(1, range(4, NK))
                load_w24(1)

            # column softmax sums in [128,4] layout, reciprocal, broadcast back
            P = sb_st.tile([Q_LEN, R], bf16, tag="P" + sfx)
            for rh in range(RH):
                E = Es[rh]
                ps_l1c = ps_sm.tile([128, 4], f32, tag="ps_sm")
                for q4 in range(4):
                    nc.tensor.matmul(ps_l1c[:, q4:q4 + 1],
                                     E[:, q4 * 128:(q4 + 1) * 128], ones_qb16[:],
                                     start=True, stop=True)
                l1r = sb_rh.tile([128, 4], f32r, tag="l1r")
                with nc.allow_low_precision(reason="softmax scale in f32r"):
                    nc.vector.reciprocal(l1r[:], ps_l1c[:])
                ps_rb = ps_bc.tile([128, 512], f32, tag="ps_bc")
                for q4 in range(4):
                    nc.tensor.matmul(
                        ps_rb[0:Q_LEN, q4 * 128:(q4 + 1) * 128],
                        l1r[:, q4:q4 + 1].broadcast_to([128, Q_LEN]),
                        ident[:], start=True, stop=True)
                nc.vector.tensor_tensor(P[:, rh * 512:(rh + 1) * 512],
                                        E[:], ps_rb[0:Q_LEN], Alu.mult)

            # ---------- o1 / XO interleaved with W1-parts of early groups ---
            XO = sb_xo.tile([128, NK, R], bf16, tag="xo")
            oT = sb_x.tile([128, NK, R], bf16, tag="xt")
            w13s = w13_t[s]
            group_ps = {}

            def o1_pair(rh, c0):
                sl = slice(rh * 512, (rh + 1) * 512)
                for c in (c0, c0 + 1):
                    ps_o1 = ps_att.tile([128, 512], f32, tag="ps_att")
                    nc.tensor.matmul(ps_o1[:], mN[:, c * 128:(c + 1) * 128],
                                     P[:, sl], start=True, stop=True)
                    nc.vector.tensor_tensor(XO[:, c, sl], Xt[:, c, sl],
                                            ps_o1[:], Alu.mult)

            def xpart(j, rh, cs):
                sl = slice(rh * 512, (rh + 1) * 512)
                if (j, rh) not in group_ps:
                    group_ps[(j, rh)] = ps_big.tile([128, 512], f32,
                                                    tag="ps_big", name="ps_ab")
                ps_ab = group_ps[(j, rh)]
                for c in cs:
                    nc.tensor.matmul(ps_ab[:], w13s[j][:, 0, c], Xt[:, c, sl],
                                     start=(c == 0), stop=False)

            def xoclose(j, rh, fuse_r64=False, Rsb=None):
                sl = slice(rh * 512, (rh + 1) * 512)
                ps_ab = group_ps.pop((j, rh))
                for c in range(NK):
                    nc.tensor.matmul(ps_ab[:], w13s[j][:, 1, c], XO[:, c, sl],
                                     start=False,
                                     stop=(c == NK - 1 and not fuse_r64))
                if fuse_r64:
                    nc.tensor.matmul(ps_ab[:], Rsb[:, j * 128:(j + 1) * 128],
                                     P[:, sl], start=False, stop=True)
                nc.scalar.activation(oT[:, j, sl], ps_ab[:], AF.Copy)

            o1_pair(0, 0)
            xpart(0, 0, range(0, 4))
            o1_pair(0, 2)
            xpart(0, 0, range(4, 8))
            o1_pair(0, 4)
            xpart(1, 0, range(0, 4))
            o1_pair(0, 6)
            xpart(1, 0, range(4, 8))
            o1_pair(1, 0)
            xpart(0, 1, range(0, 4))
            o1_pair(1, 2)
            xpart(0, 1, range(4, 8))
            o1_pair(1, 4)
            xoclose(0, 0)
            o1_pair(1, 6)
            xoclose(1, 0)
            xoclose(0, 1)

            def big_group(j, rh, fuse_r64=False, Rsb=None):
                xpart(j, rh, range(NK))
                xoclose(j, rh, fuse_r64=fuse_r64, Rsb=Rsb)

            big_group(1, 1)

            # ---------- weight-two stats (PE busy on big blocks) ----------
            vh = sb_st.tile([128, 2 * NK], f32, tag="vh" + sfx)
            l2col = sb_st.tile([1, 2], f32, tag="l2col" + sfx)

            def stats_rh(rh):
                e2 = sb_rh.tile([1, 512], f32r, tag="e2")
                nc.vector.tensor_tensor(e2[:], mxs[rh][0:1], eids[rh][:], Alu.mult)
                nc.vector.reduce_sum(l2col[:, rh:rh + 1], e2[:].bitcast(f32), AX.X)
                ps_eb = ps_bc.tile([128, 512], f32, tag="ps_bc")
                nc.tensor.matmul(ps_eb[:], ones_row[:], e2[:], start=True, stop=True)
                e2b = sb_rh.tile([128, 512], bf16, tag="e2b")
                nc.scalar.activation(e2b[:], ps_eb[:], AF.Copy)
                scrv = sb_rh.tile([128, 512], bf16, tag="scrv")
                sl = slice(rh * 512, (rh + 1) * 512)
                for c in range(NK):
                    nc.vector.scalar_tensor_tensor(
                        scrv[:], Xt[:, c, sl], 1.0, e2b[:],
                        Alu.mult, Alu.mult,
                        accum_out=vh[:, 2 * c + rh:2 * c + rh + 1])

            big_group(2, 0)
            stats_rh(0)
            big_group(2, 1)
            stats_rh(1)
            big_group(3, 0)

            l2 = sb_st.tile([1, 1], f32, tag="l2" + sfx)
            nc.vector.reduce_sum(l2[:], l2col[:], AX.X)
            vsum = sb_st.tile([128, NK], f32, tag="vsum" + sfx)
            vh3 = vh[:].rearrange("p (c t) -> p c t", t=2)
            nc.vector.tensor_tensor(vsum[:], vh3[:, :, 0], vh3[:, :, 1], Alu.add)
            colsb = sb_st.tile([128, 16], f32, tag="colsb" + sfx)
            nc.vector.memset(colsb[:], 0.0)
            nc.vector.tensor_copy(colsb[:, 0:NK], vsum[:])
            nc.vector.tensor_copy(colsb[0:1, NK:NK + 1], l2[:])
            nc.vector.tensor_copy(colsb[0:1, NK + 1:NK + 2], l2[:])
            cin = dram.tile([128, 16], f32, tag="cin" + sfx)
            cout = dram.tile([128, 16], f32, tag="cout" + sfx)
            nc.sync.dma_start(cin[:], colsb[:])
            nc.gpsimd.collective_compute(
                "AllReduce", Alu.add,
                replica_groups=[[0, 1], [2, 3], [4, 5], [6, 7]],
                ins=[cin[:].opt()], outs=[cout[:].opt()])
            colg = sb_st.tile([128, 16], f32, tag="colg" + sfx)
            nc.sync.dma_start(colg[:], cout[:])

            big_group(3, 1)
            big_group(4, 0)
            big_group(4, 1)

            # prefetch next stage / final-linear weights + prep
            if s == 1:
                load_w13(2, range(NK))
                prep[2] = prep_stage(2)
            else:
                for j2 in range(16):
                    wmj = sb_wm.tile([128, NK, 128], bf16, tag="wm")
                    nc.sync.dma_start(
                        wmj[:], wmt_ap[:, j2 * 1024:(j2 + 1) * 1024]
                        .rearrange("p (c m) -> p c m", c=NK))
                    wm_tiles.append(wmj)
                nc.sync.dma_start(bcol_all[:], bmap_ap[:])

            # ---------- collective-dependent tail, PE kept dense ----------
            # W2-part of R = M^T(W2 + t*W4) first (no collective dep),
            # groups j5 next, then the t-dependent w4 half, then the last
            # two j-blocks with the rank-64 correction fused in-group while
            # the DVE applies the correction to j0..j5.
            w24s = w24_t[s]
            ps_r = []
            for hf in range(2):
                ps_ri = ps_att.tile([128, 512], f32, tag="ps_att")
                ps_r.append(ps_ri)
                slh = slice(hf * 512, (hf + 1) * 512)
                for c in range(NK):
                    nc.tensor.matmul(ps_ri[0:Q_LEN], mT[:, c], w24s[c][:, 0, slh],
                                     start=(c == 0), stop=False)
            linv = sb_st.tile([1, 2], f32r, tag="linv" + sfx)
            with nc.allow_low_precision(reason="weight-two scale in f32r"):
                nc.vector.reciprocal(linv[:], colg[0:1, NK:NK + 2])
            ps_tb = ps_sm.tile([128, 4], f32, tag="ps_sm")
            nc.tensor.matmul(ps_tb[:, 0:2], ones_row[:], linv[:], start=True, stop=True)
            tvec = sb_st.tile([128, NK], f32, tag="tvec" + sfx)
            nc.vector.tensor_scalar(tvec[:], colg[:, 0:NK], ps_tb[:, 0:1], None, Alu.mult)
            w4sc = []
            for c in range(NK):
                w4c = sb_ws.tile([128, D], bf16, tag="w4sc")
                nc.vector.tensor_scalar(w4c[:], w24s[c][:, 1], tvec[:, c:c + 1],
                                        None, Alu.mult)
                w4sc.append(w4c)

            big_group(5, 0)
            big_group(5, 1)

            for hf in range(2):
                slh = slice(hf * 512, (hf + 1) * 512)
                for c in range(NK):
                    nc.tensor.matmul(ps_r[hf][0:Q_LEN], mT[:, c], w4sc[c][:, slh],
                                     start=False, stop=(c == NK - 1))
            Rsb = sb_st.tile([Q_LEN, D], bf16, tag="Rsb" + sfx)
            for hf in range(2):
                nc.scalar.activation(Rsb[:, hf * 512:(hf + 1) * 512],
                                     ps_r[hf][0:Q_LEN], AF.Copy)

            # rank-64 correction: j6/j7 fused in-group; j0..j5 via psum + add
            def r64(j, rh):
                sl = slice(rh * 512, (rh + 1) * 512)
                ps_c = ps_att.tile([128, 512], f32, tag="ps_att")
                nc.tensor.matmul(ps_c[:], Rsb[:, j * 128:(j + 1) * 128], P[:, sl],
                                 start=True, stop=True)
                nc.vector.tensor_tensor(oT[:, j, sl], oT[:, j, sl], ps_c[:], Alu.add)

            big_group(6, 0, fuse_r64=True, Rsb=Rsb)
            r64(0, 0)
            r64(1, 0)
            big_group(6, 1, fuse_r64=True, Rsb=Rsb)
            r64(2, 0)
            r64(3, 0)
            big_group(7, 0, fuse_r64=True, Rsb=Rsb)
            r64(4, 0)
            r64(5, 0)
            big_group(7, 1, fuse_r64=True, Rsb=Rsb)
            for j in range(6):
                r64(j, 1)
            return oT

        o1T = run_stage(1, xt0)
        load_w24(2)
        o2T = run_stage(2, o1T)

        # ---------- final linear (transposed): outT = w_mapT.T @ o2T + b ----
        for j2 in range(16):
            wmj = wm_tiles[j2]
            for rh in range(RH):
                sl = slice(rh * 512, (rh + 1) * 512)
                ps_f = ps_big.tile([128, 512], f32, tag="ps_big")
                for c in range(NK):
                    nc.tensor.matmul(ps_f[:], wmj[:, c], o2T[:, c, sl],
                                     start=(c == 0), stop=(c == NK - 1))
                outsb = sb_ws.tile([128, 512], bf16, tag="outsb")
                if (j2 + rh) % 2 == 0:
                    nc.scalar.activation(outsb[:], ps_f[:], AF.Identity,
                                         bias=bcol_all[:, j2:j2 + 1], scale=1.0)
                else:
                    nc.vector.tensor_scalar(outsb[:], ps_f[:],
                                            bcol_all[:, j2:j2 + 1], None, Alu.add)
                nc.sync.dma_start(out_ap[j2 * 128:(j2 + 1) * 128, sl], outsb[:])

    nc.compile()
    return nc


def _get_nc():
    global _CACHED_NC
    if _CACHED_NC is None:
        _CACHED_NC = _build_nc()
    return _CACHED_NC


def _bf(a):
    return np.ascontiguousarray(np.asarray(a, dtype=np.float32).astype(ml_dtypes.bfloat16))


def _shard_inputs(inputs):
    """Build the 8 per-core input maps (pure layout work, no arithmetic)."""
    x = np.asarray(inputs["ctx_features"], dtype=np.float32)
    q1 = np.asarray(inputs["sub_q1_features"], dtype=np.float32)
    q2 = np.asarray(inputs["sub_q2_features"], dtype=np.float32)
    k1 = np.ascontiguousarray(np.asarray(inputs["sub_q1_attn_mask"], dtype=np.int32))
    k2 = np.ascontiguousarray(np.asarray(inputs["sub_q2_attn_mask"], dtype=np.int32))

    def wpack13(w_out):
        # w_out [D, 4D] -> wb = w_out.T [4D, D]; W_k = wb[kD:(k+1)D]
        wb = np.asarray(w_out, dtype=np.float32).T
        W1, W3 = wb[0:D], wb[2 * D:3 * D]

        def v(W):  # [D_in, D_out] -> [c, p, j, m]
            return W.reshape(NK, 128, NK, 128)
        pk = np.stack([v(W1), v(W3)], axis=0)  # [t, c, p, j, m]
        pk = pk.transpose(2, 3, 0, 1, 4)       # [p, j, t, c, m]
        return _bf(pk.reshape(128, NK * 2 * NK * 128))

    def wpack24(w_out):
        wb = np.asarray(w_out, dtype=np.float32).T
        W2, W4 = wb[D:2 * D], wb[3 * D:4 * D]
        pk = np.stack([W2.reshape(NK, 128, D), W4.reshape(NK, 128, D)], axis=0)
        pk = pk.transpose(2, 1, 0, 3)          # [p, c, t, m]
        return _bf(pk.reshape(128, NK * 2 * D))

    wmT = np.asarray(inputs["w_map"], dtype=np.float32).T  # [D, 2D]
    wmt = wmT.reshape(NK, 128, 16, 128).transpose(1, 2, 0, 3)
    wmt = _bf(wmt.reshape(128, 16 * NK * 128))
    bmap = np.ascontiguousarray(
        np.asarray(inputs["b_map"], dtype=np.float32).reshape(16, 128).T)

    def ptile(vec_list, dtype):  # [D] vectors -> [128, NK*k] p-major
        v = np.stack([np.asarray(c, dtype=np.float32) for c in vec_list], axis=-1)
        k = v.shape[-1]
        out = v.reshape(NK, 128, k).transpose(1, 0, 2).reshape(128, NK * k)
        if dtype == "bf16":
            return _bf(out)
        return np.ascontiguousarray(out)

    stage_common = {
        "vecb1": ptile([inputs["w_in1"], inputs["w_mem1"]], "bf16"),
        "vecb2": ptile([inputs["w_in2"], inputs["w_mem2"]], "bf16"),
        "sclf1": ptile([inputs["scale1"]], "f32"),
        "sclf2": ptile([inputs["scale2"]], "f32"),
        "w13_1": wpack13(inputs["w_out1"]), "w13_2": wpack13(inputs["w_out2"]),
        "w24_1": wpack24(inputs["w_out1"]), "w24_2": wpack24(inputs["w_out2"]),
        "wmt": wmt, "bmap": bmap,
        "ident": np.ascontiguousarray(np.eye(128, dtype=np.float32)),
    }

    in_maps = []
    for core in range(N_CORES):
        b, h = divmod(core, 2)
        xT = x[b, h * R:(h + 1) * R, :].T  # [D, R]
        xt_tile = _bf(xT.reshape(NK, 128, R).transpose(1, 0, 2).reshape(128, NK * R))
        m = {}
        for s, q, kk in ((1, q1, k1), (2, q2, k2)):
            mT = q[b].T  # [D, Q]
            m[f"m{s}t"] = _bf(
                mT.reshape(NK, 128, Q_LEN).transpose(1, 0, 2).reshape(128, NK * Q_LEN))
            m[f"m{s}n"] = _bf(q[b])
            m[f"mask{s}"] = np.ascontiguousarray(kk[b].reshape(Q_LEN, 1))
        in_maps.append({"xt": xt_tile, **m, **stage_common})
    return in_maps


def _gather_outputs(results):
    out = np.empty((B, C_LEN, D2), dtype=np.float32)
    for core in range(N_CORES):
        b, h = divmod(core, 2)
        out[b, h * R:(h + 1) * R, :] = results[core]["out"].T.astype(np.float32)
    return out


def kernel(**inputs):
    nc = _get_nc()
    in_maps = _shard_inputs(inputs)
    last_err = None
    for _attempt in range(3):
        try:
            res = run_bass_kernel_spmd(nc, in_maps, core_ids=list(range(N_CORES)))
            return _gather_outputs(res.results)
        except Exception as e:  # transient device errors: retry
            last_err = e
    raise last_err


# revision 21
# speedup vs baseline: 1.6502x; 1.0138x over previous
"""Trainium2 Bass kernel for nn_NewModel_42356967473589 (dense_transformer).

Model: two BiAttention blocks + final linear mapping.
  o = BiAttn(ctx, q1) ; o = BiAttn(o, q2) ; out = o @ w_map.T + b_map

Sharding: 8 cores = (batch b in 0..3) x (context half h in 0..1).
Each core owns 1024 context rows of one batch. All compute is row-local
except the softmax-over-context (weight_two); its (sum-exp, weighted-sum)
stats are combined across the pair of cores sharing a batch via a tiny
pairwise AllReduce, overlapped with the large matmuls.

Math restructure (per stage, X = stage input [C,D], M = memory [Q,D]):
  out = X@W1 + o1@W2 + (X*o1)@W3 + (t*o1)@W4      (W_k = w_out[:, kD:(k+1)D].T)
  o1 = P@M (rank Q=64), t broadcast over rows =>
  o1@W2 + (t*o1)@W4 = P @ (M @ (W2 + t*W4))        (rank-64 path)

v3: all heavy matmul operands bf16 (psum fp32); softmax reciprocal in a
[128,4] column layout (DVE reciprocal is 8 cyc/elem/lane, so [1,512] on
one partition costs 4.3us vs ~0.2us here), row-broadcast back via a
stride-0-lhsT identity matmul.  PE queue is kept dense end-to-end (HAM
clock gate re-throttles after idle): o1 matmuls interleave with the
W1-parts of the first output groups, the rank-64 correction of the last
two j-blocks rides inside their psum accumulation groups, and weight
DMAs are dependency-gated on the gpsimd queue so the stage-1 input
transfer gets full HBM bandwidth at startup.
"""

import numpy as np
import ml_dtypes

import concourse.bacc as bacc
import concourse.tile as tile
from concourse import mybir
from concourse.bass_utils import run_bass_kernel_spmd
from contextlib import ExitStack
import bass_rust

f32 = mybir.dt.float32
f32r = mybir.dt.float32r
bf16 = mybir.dt.bfloat16
i32 = mybir.dt.int32
Alu = mybir.AluOpType
AF = bass_rust.ActivationFunctionType
AX = bass_rust.AxisListType
RedOp = bass_rust.ReduceOp

B, C_LEN, Q_LEN, D = 4, 2048, 64, 1024
N_CORES = 8
R = C_LEN // 2          # rows per core
NK = D // 128           # contraction chunks
RH = R // 512           # row halves (moving-dim tiles)
D2 = 2 * D
NEGBIG = 10000.0

_CACHED_NC = None


def _build_nc():
    nc = bacc.Bacc("TRN2", target_bir_lowering=False, debug=False,
                   num_devices=N_CORES)

    # ---- per-core DRAM I/O (host pre-tiled layouts, see _shard_inputs) ----
    xt_ap = nc.dram_tensor("xt", [128, NK * R], bf16, kind="ExternalInput").ap()
    m_t = [nc.dram_tensor(f"m{s}t", [128, NK * Q_LEN], bf16, kind="ExternalInput").ap() for s in (1, 2)]
    m_n = [nc.dram_tensor(f"m{s}n", [Q_LEN, D], bf16, kind="ExternalInput").ap() for s in (1, 2)]
    vecb = [nc.dram_tensor(f"vecb{s}", [128, NK * 2], bf16, kind="ExternalInput").ap() for s in (1, 2)]
    sclf = [nc.dram_tensor(f"sclf{s}", [128, NK], f32, kind="ExternalInput").ap() for s in (1, 2)]
    msk = [nc.dram_tensor(f"mask{s}", [Q_LEN, 1], i32, kind="ExternalInput").ap() for s in (1, 2)]
    w13 = [nc.dram_tensor(f"w13_{s}", [128, NK * 2 * NK * 128], bf16, kind="ExternalInput").ap() for s in (1, 2)]
    w24 = [nc.dram_tensor(f"w24_{s}", [128, NK * 2 * D], bf16, kind="ExternalInput").ap() for s in (1, 2)]
    wmt_ap = nc.dram_tensor("wmt", [128, 16 * NK * 128], bf16, kind="ExternalInput").ap()
    bmap_ap = nc.dram_tensor("bmap", [128, 16], f32, kind="ExternalInput").ap()
    ident_ap = nc.dram_tensor("ident", [128, 128], f32r, kind="ExternalInput").ap()
    out_ap = nc.dram_tensor("out", [D2, R], bf16, kind="ExternalOutput").ap()

    with tile.TileContext(nc) as tc, ExitStack() as ctx:
        sb_x = ctx.enter_context(tc.tile_pool(name="sb_x", bufs=2))
        sb_xo = ctx.enter_context(tc.tile_pool(name="sb_xo", bufs=1))
        sb_w13 = ctx.enter_context(tc.tile_pool(name="sb_w13", bufs=16))
        sb_w24 = ctx.enter_context(tc.tile_pool(name="sb_w24", bufs=8))
        sb_wm = ctx.enter_context(tc.tile_pool(name="sb_wm", bufs=8))
        sb_ws = ctx.enter_context(tc.tile_pool(name="sb_ws", bufs=2))
        sb_st = ctx.enter_context(tc.tile_pool(name="sb_st", bufs=1))
        sb_rh = ctx.enter_context(tc.tile_pool(name="sb_rh", bufs=2))
        ps_att = ctx.enter_context(tc.tile_pool(name="ps_att", bufs=3, space="PSUM"))
        ps_big = ctx.enter_context(tc.tile_pool(name="ps_big", bufs=3, space="PSUM"))
        ps_sm = ctx.enter_context(tc.tile_pool(name="ps_sm", bufs=1, space="PSUM"))
        ps_bc = ctx.enter_context(tc.tile_pool(name="ps_bc", bufs=1, space="PSUM"))
        dram = ctx.enter_context(tc.tile_pool(name="dram", bufs=2, space="DRAM"))

        # ---- constants ----
        ones_row = sb_st.tile([1, 128], f32r, tag="ones_row")
        nc.vector.memset(ones_row[:].bitcast(f32), 1.0)
        ones_qb16 = sb_st.tile([Q_LEN, 1], bf16, tag="ones_qb16")
        nc.vector.memset(ones_qb16[:], 1.0)

        # ---- stage-1 input + const DMAs (sync queue: small stuff only) ----
        def load_stage_consts(s):
            vb = sb_st.tile([128, NK, 2], bf16, tag=f"vb{s}")
            nc.sync.dma_start(vb[:], vecb[s - 1][:].rearrange("p (c k) -> p c k", c=NK))
            sf = sb_st.tile([128, NK], f32, tag=f"sf{s}")
            nc.sync.dma_start(sf[:], sclf[s - 1][:])
            mT = sb_st.tile([128, NK, Q_LEN], bf16, tag=f"mT{s}")
            nc.sync.dma_start(mT[:], m_t[s - 1][:].rearrange("p (c q) -> p c q", c=NK))
            mN = sb_st.tile([Q_LEN, D], bf16, tag=f"mN{s}")
            nc.sync.dma_start(mN[:], m_n[s - 1][:])
            mask_i = sb_st.tile([Q_LEN, 1], i32, tag=f"mask_i{s}")
            nc.sync.dma_start(mask_i[:], msk[s - 1][:])
            return vb, sf, mT, mN, mask_i

        consts = {1: load_stage_consts(1)}
        ident = sb_st.tile([128, 128], f32r, tag="ident")
        nc.sync.dma_start(ident[:], ident_ap[:])
        consts[2] = load_stage_consts(2)

        # xt first on gpsimd so it gets HBM bandwidth; chunked so the first
        # scores matmuls can start while later chunks are still in flight.
        # Weights follow, the late ones gated behind the (E-dependent)
        # gpsimd max ops.
        xt0 = sb_x.tile([128, NK, R], bf16, tag="xt")
        for c in range(NK):
            nc.gpsimd.dma_start(xt0[:, c], xt_ap[:, c * R:(c + 1) * R])

        w13_t = {1: [], 2: []}
        w24_t = {1: [], 2: []}

        def load_w13(s, js, eng=None):
            eng = eng or nc.gpsimd
            for j in js:
                w13j = sb_w13.tile([128, 2, NK, 128], bf16, tag="w13")
                eng.dma_start(
                    w13j[:], w13[s - 1][:, j * 2048:(j + 1) * 2048]
                    .rearrange("p (t c m) -> p t c m", t=2, c=NK))
                w13_t[s].append(w13j)

        def load_w24(s, eng=None):
            eng = eng or nc.gpsimd
            for c in range(NK):
                w24c = sb_w24.tile([128, 2, D], bf16, tag="w24")
                eng.dma_start(
                    w24c[:], w24[s - 1][:, c * 2 * D:(c + 1) * 2 * D]
                    .rearrange("p (t m) -> p t m", t=2))
                w24_t[s].append(w24c)

        load_w13(1, range(0, 4))
        load_w13(1, range(4, NK), eng=nc.sync)
        load_w24(1, eng=nc.sync)

        # ---- per-stage prep: mst, memory_dot, mbias (runs early) ----
        def prep_stage(s):
            vb, sf, mT, mN, mask_i = consts[s]
            sfx = f"_s{s}"
            mst = sb_st.tile([128, NK, Q_LEN + 1], bf16, tag="mst" + sfx)
            nc.vector.tensor_copy(mst[:, :, 0:Q_LEN], mT[:])
            nc.vector.tensor_copy(mst[:, :, Q_LEN:Q_LEN + 1], vb[:, :, 0:1])
            for c in range(NK):
                nc.vector.tensor_scalar(mst[:, c, 0:Q_LEN], mst[:, c, 0:Q_LEN],
                                        sf[:, c:c + 1], None, Alu.mult)
            ps_md = ps_sm.tile([128, 4], f32, tag="ps_sm")
            for c in range(NK):
                nc.tensor.matmul(ps_md[0:Q_LEN, 0:1], mT[:, c], vb[:, c, 1:2],
                                 start=(c == 0), stop=(c == NK - 1))
            maskf = sb_st.tile([Q_LEN, 1], f32, tag="maskf" + sfx)
            nc.vector.tensor_copy(maskf[:], mask_i[:])
            mbias = sb_st.tile([Q_LEN, 1], f32, tag="mbias" + sfx)
            nc.vector.tensor_scalar(mbias[:], maskf[:], NEGBIG, -NEGBIG, Alu.mult, Alu.add)
            nc.vector.tensor_tensor(mbias[:], mbias[:], ps_md[0:Q_LEN, 0:1], Alu.add)
            return mst, mbias

        prep = {1: prep_stage(1)}
        wm_tiles = []
        bcol_all = sb_st.tile([128, 16], f32, tag="bcol_all")

        def run_stage(s, Xt):
            """One BiAttention stage; returns o^T tile [128, NK, R] bf16."""
            sfx = f"_s{s}"
            vb, sf, mT, mN, mask_i = consts[s]
            mst, mbias = prep[s]

            # ---------- scores for both row-halves ----------
            Es, eids = [], []
            for rh in range(RH):
                sl = slice(rh * 512, (rh + 1) * 512)
                ps_sc = ps_att.tile([Q_LEN + 1, 512], f32, tag="ps_att")
                for c in range(NK):
                    nc.tensor.matmul(ps_sc[:], mst[:, c], Xt[:, c, sl],
                                     start=(c == 0), stop=(c == NK - 1))
                E = sb_rh.tile([Q_LEN, 512], bf16, tag="E")
                nc.scalar.activation(E[:], ps_sc[0:Q_LEN], AF.Exp,
                                     bias=mbias[:], scale=1.0)
                eid = sb_rh.tile([1, 512], f32, tag="eid")
                nc.scalar.activation(eid[:], ps_sc[Q_LEN:Q_LEN + 1], AF.Exp)
                Es.append(E)
                eids.append(eid)

            # gpsimd max over q (for weight_two)
            mxs = []
            for rh in range(RH):
                mx = sb_rh.tile([Q_LEN, 512], f32, tag="mx")
                nc.gpsimd.partition_all_reduce(mx[:], Es[rh][:], Q_LEN, RedOp.max)
                mxs.append(mx)

            # column softmax sums in [128,4] layout, reciprocal, broadcast back
            P = sb_st.tile([Q_LEN, R], bf16, tag="P" + sfx)
            for rh in range(RH):
                E = Es[rh]
                ps_l1c = ps_sm.tile([128, 4], f32, tag="ps_sm")
                for q4 in range(4):
                    nc.tensor.matmul(ps_l1c[:, q4:q4 + 1],
                                     E[:, q4 * 128:(q4 + 1) * 128], ones_qb16[:],
                                     start=True, stop=True)
                l1r = sb_rh.tile([128, 4], f32r, tag="l1r")
                with nc.allow_low_precision(reason="softmax scale in f32r"):
                    nc.vector.reciprocal(l1r[:], ps_l1c[:])
                ps_rb = ps_bc.tile([128, 512], f32, tag="ps_bc")
                for q4 in range(4):
                    nc.tensor.matmul(
                        ps_rb[0:Q_LEN, q4 * 128:(q4 + 1) * 128],
                        l1r[:, q4:q4 + 1].broadcast_to([128, Q_LEN]),
                        ident[:], start=True, stop=True)
                nc.vector.tensor_tensor(P[:, rh * 512:(rh + 1) * 512],
                                        E[:], ps_rb[0:Q_LEN], Alu.mult)

            # weight_two per-column weights e2 (early, so the collective can
            # trigger as soon as possible; broadcast + partial sums on gpsimd)
            vh = sb_st.tile([128, 2 * NK], f32, tag="vh" + sfx)
            l2col = sb_st.tile([1, 2], f32, tag="l2col" + sfx)
            e2bs = []
            for rh in range(RH):
                e2 = sb_rh.tile([1, 512], bf16, tag="e2")
                nc.vector.tensor_tensor(e2[:], mxs[rh][0:1], eids[rh][:], Alu.mult)
                nc.vector.reduce_sum(l2col[:, rh:rh + 1], e2[:], AX.X)
                e2b = sb_rh.tile([128, 512], bf16, tag="e2b")
                nc.gpsimd.partition_broadcast(e2b[:], e2[:], 128)
                e2bs.append(e2b)

            # ---------- o1 / XO interleaved with W1-parts of early groups ---
            XO = sb_xo.tile([128, NK, R], bf16, tag="xo")
            oT = sb_x.tile([128, NK, R], bf16, tag="xt")
            w13s = w13_t[s]
            group_ps = {}

            def o1_pair(rh, c0):
                sl = slice(rh * 512, (rh + 1) * 512)
                for c in (c0, c0 + 1):
                    ps_o1 = ps_att.tile([128, 512], f32, tag="ps_att")
                    nc.tensor.matmul(ps_o1[:], mN[:, c * 128:(c + 1) * 128],
                                     P[:, sl], start=True, stop=True)
                    if c % 2 == 0:
                        nc.vector.tensor_tensor(XO[:, c, sl], Xt[:, c, sl],
                                                ps_o1[:], Alu.mult)
                    else:
                        # scalar-engine copy frees DVE (2x mode on bf16 pair)
                        o1s = sb_rh.tile([128, 512], bf16, tag="o1s")
                        nc.scalar.activation(o1s[:], ps_o1[:], AF.Copy)
                        nc.vector.tensor_tensor(XO[:, c, sl], Xt[:, c, sl],
                                                o1s[:], Alu.mult)

            def xpart(j, rh, cs):
                sl = slice(rh * 512, (rh + 1) * 512)
                if (j, rh) not in group_ps:
                    group_ps[(j, rh)] = ps_big.tile([128, 512], f32,
                                                    tag="ps_big", name="ps_ab")
                ps_ab = group_ps[(j, rh)]
                for c in cs:
                    nc.tensor.matmul(ps_ab[:], w13s[j][:, 0, c], Xt[:, c, sl],
                                     start=(c == 0), stop=False)

            def xoclose(j, rh, fuse_r64=False, Rsb=None):
                sl = slice(rh * 512, (rh + 1) * 512)
                ps_ab = group_ps.pop((j, rh))
                for c in range(NK):
                    nc.tensor.matmul(ps_ab[:], w13s[j][:, 1, c], XO[:, c, sl],
                                     start=False,
                                     stop=(c == NK - 1 and not fuse_r64))
                if fuse_r64:
                    nc.tensor.matmul(ps_ab[:], Rsb[:, j * 128:(j + 1) * 128],
                                     P[:, sl], start=False, stop=True)
                nc.scalar.activation(oT[:, j, sl], ps_ab[:], AF.Copy)

            o1_pair(0, 0)
            xpart(0, 0, range(0, 4))
            o1_pair(0, 2)
            xpart(0, 0, range(4, 8))
            o1_pair(0, 4)
            xpart(1, 0, range(0, 4))
            o1_pair(0, 6)
            xpart(1, 0, range(4, 8))
            o1_pair(1, 0)
            xpart(0, 1, range(0, 4))
            o1_pair(1, 2)
            xpart(0, 1, range(4, 8))
            o1_pair(1, 4)
            xoclose(0, 0)
            o1_pair(1, 6)
            xoclose(1, 0)
            xoclose(0, 1)

            def big_group(j, rh, fuse_r64=False, Rsb=None):
                xpart(j, rh, range(NK))
                xoclose(j, rh, fuse_r64=fuse_r64, Rsb=Rsb)

            big_group(1, 1)

            # ---------- weight-two stats (PE busy on big blocks) ----------
            # partial sums v = X^T e2: even chunks on gpsimd (idle anyway),
            # odd chunks on DVE (after XO in its queue).
            def stats_rh(rh):
                scrv = sb_rh.tile([128, 512], bf16, tag="scrv")
                sl = slice(rh * 512, (rh + 1) * 512)
                for c in range(NK):
                    nc.vector.scalar_tensor_tensor(
                        scrv[:], Xt[:, c, sl], 1.0, e2bs[rh][:],
                        Alu.mult, Alu.mult,
                        accum_out=vh[:, 2 * c + rh:2 * c + rh + 1])

            big_group(2, 0)
            stats_rh(0)
            big_group(2, 1)
            stats_rh(1)
            big_group(3, 0)

            # W2-part of R = M^T W2 (no collective dep) — early PE filler
            w24s = w24_t[s]
            ps_r = []
            for hf in range(2):
                ps_ri = ps_att.tile([128, 512], f32, tag="ps_att")
                ps_r.append(ps_ri)
                slh = slice(hf * 512, (hf + 1) * 512)
                for c in range(NK):
                    nc.tensor.matmul(ps_ri[0:Q_LEN], mT[:, c], w24s[c][:, 0, slh],
                                     start=(c == 0), stop=False)

            l2 = sb_st.tile([1, 1], f32, tag="l2" + sfx)
            nc.vector.reduce_sum(l2[:], l2col[:], AX.X)
            vsum = sb_st.tile([128, NK], f32, tag="vsum" + sfx)
            vh3 = vh[:].rearrange("p (c t) -> p c t", t=2)
            nc.vector.tensor_tensor(vsum[:], vh3[:, :, 0], vh3[:, :, 1], Alu.add)
            colsb = sb_st.tile([128, 16], f32, tag="colsb" + sfx)
            nc.vector.memset(colsb[:], 0.0)
            nc.vector.tensor_copy(colsb[:, 0:NK], vsum[:])
            nc.vector.tensor_copy(colsb[0:1, NK:NK + 1], l2[:])
            nc.vector.tensor_copy(colsb[0:1, NK + 1:NK + 2], l2[:])
            cin = dram.tile([128, 16], f32, tag="cin" + sfx)
            cout = dram.tile([128, 16], f32, tag="cout" + sfx)
            nc.sync.dma_start(cin[:], colsb[:])
            nc.gpsimd.collective_compute(
                "AllReduce", Alu.add,
                replica_groups=[[0, 1], [2, 3], [4, 5], [6, 7]],
                ins=[cin[:].opt()], outs=[cout[:].opt()])
            colg = sb_st.tile([128, 16], f32, tag="colg" + sfx)
            nc.sync.dma_start(colg[:], cout[:])

            big_group(3, 1)
            big_group(4, 0)
            big_group(4, 1)

            # prefetch next stage / final-linear weights + prep
            if s == 1:
                load_w13(2, range(NK))
                prep[2] = prep_stage(2)
            else:
                for j2 in range(16):
                    wmj = sb_wm.tile([128, NK, 128], bf16, tag="wm")
                    nc.sync.dma_start(
                        wmj[:], wmt_ap[:, j2 * 1024:(j2 + 1) * 1024]
                        .rearrange("p (c m) -> p c m", c=NK))
                    wm_tiles.append(wmj)
                nc.sync.dma_start(bcol_all[:], bmap_ap[:])

            # ---------- collective-dependent tail, PE kept dense ----------
            linv = sb_st.tile([1, 2], f32r, tag="linv" + sfx)
            with nc.allow_low_precision(reason="weight-two scale in f32r"):
                nc.vector.reciprocal(linv[:], colg[0:1, NK:NK + 2])
            ps_tb = ps_sm.tile([128, 4], f32, tag="ps_sm")
            nc.tensor.matmul(ps_tb[:, 0:2], ones_row[:], linv[:], start=True, stop=True)
            tvec = sb_st.tile([128, NK], f32, tag="tvec" + sfx)
            nc.vector.tensor_scalar(tvec[:], colg[:, 0:NK], ps_tb[:, 0:1], None, Alu.mult)
            w4sc = []
            for c in range(NK):
                w4c = sb_ws.tile([128, D], bf16, tag="w4sc")
                nc.vector.tensor_scalar(w4c[:], w24s[c][:, 1], tvec[:, c:c + 1],
                                        None, Alu.mult)
                w4sc.append(w4c)

            big_group(5, 0)
            big_group(5, 1)

            for hf in range(2):
                slh = slice(hf * 512, (hf + 1) * 512)
                for c in range(NK):
                    nc.tensor.matmul(ps_r[hf][0:Q_LEN], mT[:, c], w4sc[c][:, slh],
                                     start=False, stop=(c == NK - 1))
            Rsb = sb_st.tile([Q_LEN, D], bf16, tag="Rsb" + sfx)
            for hf in range(2):
                nc.scalar.activation(Rsb[:, hf * 512:(hf + 1) * 512],
                                     ps_r[hf][0:Q_LEN], AF.Copy)

            # rank-64 correction: j6/j7 fused in-group; j0..j5 via psum + add
            def r64(j, rh):
                sl = slice(rh * 512, (rh + 1) * 512)
                ps_c = ps_att.tile([128, 512], f32, tag="ps_att")
                nc.tensor.matmul(ps_c[:], Rsb[:, j * 128:(j + 1) * 128], P[:, sl],
                                 start=True, stop=True)
                nc.vector.tensor_tensor(oT[:, j, sl], oT[:, j, sl], ps_c[:], Alu.add)

            big_group(6, 0, fuse_r64=True, Rsb=Rsb)
            r64(0, 0)
            r64(1, 0)
            big_group(6, 1, fuse_r64=True, Rsb=Rsb)
            r64(2, 0)
            r64(3, 0)
            big_group(7, 0, fuse_r64=True, Rsb=Rsb)
            r64(4, 0)
            r64(5, 0)
            big_group(7, 1, fuse_r64=True, Rsb=Rsb)
            for j in range(6):
                r64(j, 1)
            return oT

        o1T = run_stage(1, xt0)
        load_w24(2)
        o2T = run_stage(2, o1T)

        # ---------- final linear (transposed): outT = w_mapT.T @ o2T + b ----
        for j2 in range(16):
            wmj = wm_tiles[j2]
            for rh in range(RH):
                sl = slice(rh * 512, (rh + 1) * 512)
                ps_f = ps_big.tile([128, 512], f32, tag="ps_big")
                for c in range(NK):
                    nc.tensor.matmul(ps_f[:], wmj[:, c], o2T[:, c, sl],
                                     start=(c == 0), stop=(c == NK - 1))
                outsb = sb_ws.tile([128, 512], bf16, tag="outsb")
                if (j2 + rh) % 2 == 0:
                    nc.scalar.activation(outsb[:], ps_f[:], AF.Identity,
                                         bias=bcol_all[:, j2:j2 + 1], scale=1.0)
                else:
                    nc.vector.tensor_scalar(outsb[:], ps_f[:],
                                            bcol_all[:, j2:j2 + 1], None, Alu.add)
                nc.sync.dma_start(out_ap[j2 * 128:(j2 + 1) * 128, sl], outsb[:])

    nc.compile()
    return nc


def _get_nc():
    global _CACHED_NC
    if _CACHED_NC is None:
        _CACHED_NC = _build_nc()
    return _CACHED_NC


def _bf(a):
    return np.ascontiguousarray(np.asarray(a, dtype=np.float32).astype(ml_dtypes.bfloat16))


def _shard_inputs(inputs):
    """Build the 8 per-core input maps (pure layout work, no arithmetic)."""
    x = np.asarray(inputs["ctx_features"], dtype=np.float32)
    q1 = np.asarray(inputs["sub_q1_features"], dtype=np.float32)
    q2 = np.asarray(inputs["sub_q2_features"], dtype=np.float32)
    k1 = np.ascontiguousarray(np.asarray(inputs["sub_q1_attn_mask"], dtype=np.int32))
    k2 = np.ascontiguousarray(np.asarray(inputs["sub_q2_attn_mask"], dtype=np.int32))

    def wpack13(w_out):
        # w_out [D, 4D] -> wb = w_out.T [4D, D]; W_k = wb[kD:(k+1)D]
        wb = np.asarray(w_out, dtype=np.float32).T
        W1, W3 = wb[0:D], wb[2 * D:3 * D]

        def v(W):  # [D_in, D_out] -> [c, p, j, m]
            return W.reshape(NK, 128, NK, 128)
        pk = np.stack([v(W1), v(W3)], axis=0)  # [t, c, p, j, m]
        pk = pk.transpose(2, 3, 0, 1, 4)       # [p, j, t, c, m]
        return _bf(pk.reshape(128, NK * 2 * NK * 128))

    def wpack24(w_out):
        wb = np.asarray(w_out, dtype=np.float32).T
        W2, W4 = wb[D:2 * D], wb[3 * D:4 * D]
        pk = np.stack([W2.reshape(NK, 128, D), W4.reshape(NK, 128, D)], axis=0)
        pk = pk.transpose(2, 1, 0, 3)          # [p, c, t, m]
        return _bf(pk.reshape(128, NK * 2 * D))

    wmT = np.asarray(inputs["w_map"], dtype=np.float32).T  # [D, 2D]
    wmt = wmT.reshape(NK, 128, 16, 128).transpose(1, 2, 0, 3)
    wmt = _bf(wmt.reshape(128, 16 * NK * 128))
    bmap = np.ascontiguousarray(
        np.asarray(inputs["b_map"], dtype=np.float32).reshape(16, 128).T)

    def ptile(vec_list, dtype):  # [D] vectors -> [128, NK*k] p-major
        v = np.stack([np.asarray(c, dtype=np.float32) for c in vec_list], axis=-1)
        k = v.shape[-1]
        out = v.reshape(NK, 128, k).transpose(1, 0, 2).reshape(128, NK * k)
        if dtype == "bf16":
            return _bf(out)
        return np.ascontiguousarray(out)

    stage_common = {
        "vecb1": ptile([inputs["w_in1"], inputs["w_mem1"]], "bf16"),
        "vecb2": ptile([inputs["w_in2"], inputs["w_mem2"]], "bf16"),
        "sclf1": ptile([inputs["scale1"]], "f32"),
        "sclf2": ptile([inputs["scale2"]], "f32"),
        "w13_1": wpack13(inputs["w_out1"]), "w13_2": wpack13(inputs["w_out2"]),
        "w24_1": wpack24(inputs["w_out1"]), "w24_2": wpack24(inputs["w_out2"]),
        "wmt": wmt, "bmap": bmap,
        "ident": np.ascontiguousarray(np.eye(128, dtype=np.float32)),
    }

    in_maps = []
    for core in range(N_CORES):
        b, h = divmod(core, 2)
        xT = x[b, h * R:(h + 1) * R, :].T  # [D, R]
        xt_tile = _bf(xT.reshape(NK, 128, R).transpose(1, 0, 2).reshape(128, NK * R))
        m = {}
        for s, q, kk in ((1, q1, k1), (2, q2, k2)):
            mT = q[b].T  # [D, Q]
            m[f"m{s}t"] = _bf(
                mT.reshape(NK, 128, Q_LEN).transpose(1, 0, 2).reshape(128, NK * Q_LEN))
            m[f"m{s}n"] = _bf(q[b])
            m[f"mask{s}"] = np.ascontiguousarray(kk[b].reshape(Q_LEN, 1))
        in_maps.append({"xt": xt_tile, **m, **stage_common})
    return in_maps


def _gather_outputs(results):
    out = np.empty((B, C_LEN, D2), dtype=np.float32)
    for core in range(N_CORES):
        b, h = divmod(core, 2)
        out[b, h * R:(h + 1) * R, :] = results[core]["out"].T.astype(np.float32)
    return out


def kernel(**inputs):
    nc = _get_nc()
    in_maps = _shard_inputs(inputs)
    last_err = None
    for _attempt in range(3):
        try:
            res = run_bass_kernel_spmd(nc, in_maps, core_ids=list(range(N_CORES)))
            return _gather_outputs(res.results)
        except Exception as e:  # transient device errors: retry
            last_err = e
    raise last_err
